# revision 1
# baseline (speedup 1.0000x reference)
"""GAT-style message passing (DistSageConv) on 8 Trainium2 NeuronCores.

Strategy (dst-sharded, single SPMD program):
  - Core c owns dst nodes [c*1250, (c+1)*1250). dst_idx is sorted, so each
    core's edges are a contiguous slice of the edge list.
  - Host preprocessing (integer index manipulation + dtype casts only):
    per core, edges are split into 2 phases by src half (keeps gather
    indices < 32768 for int16 DMA-gather), grouped by dst, padded so that
      * every dst's per-phase edge run is a multiple of 16 (lets the
        GPSIMD ap_gather 16-partition groups deliver per-edge er/el),
      * every 16-dst window owns exactly TPW 128-edge tiles (uniform
        program structure across cores + clean PSUM accumulation groups).
  - Device per core:
      el[u] = sum_f x_src[u,f]*attn_l[f]  (f32, DVE STT with accum)
      er[v] likewise from the core's x_dst rows.
      per-edge el/er delivered via GPSIMD ap_gather from partition-
      replicated tables; z = el+er; leaky_relu; exp on ACT.
      x rows gathered in bf16 via SWDGE dma_gather (256B elements).
      S'[e, w] = (rel_dst[e]==w) * ex[e]  built on DVE (bf16).
      PE matmuls accumulate in PSUM: U += S'.T @ rows ; sum_ex += S'.T @ 1.
      out = U / max(sum_ex, tiny); DMA out per core; host concatenates.

Perf notes (2026-08-08): baseline 3.49ms -> 2.33ms by batching the el
preamble x-stream into 16-tile DMAs (was 401 x ~2us fixed-cost DMAs)
and deepening rows/sp/psum pools. A v3 rewrite (below, _run_v3) builds
dense W[128src x 1280dst] bf16 tiles per 128-src chunk via GPSIMD
local_scatter and streams x_c.T @ W_c on PE (no per-edge DMA at all;
passes at rel_err 7.2e-3). Its engine-busy totals are ~600us/engine,
but wall is 3.0-3.2ms: every GPSIMD instruction that must WAIT on an
upstream dep (gq/wt buffer released by DVE/PE) stalls a constant
~33.3us (observed in traces as exact 33,340ns APGather spacing in the
z-phase, both with and without scatter interleave). To make v3 win,
restructure so GPSIMD never blocks: pre-stage all idx tiles in SBUF,
give every ap_gather a disjoint output slice (no gq buffer reuse),
and size wt pools so local_scatter stays ahead of PE consumers.
Batching 4 er-residues per ap_gather call with gq/zf bufs=1 made it
WORSE (3.65ms): fewer instructions but each still waits, and bufs=1
serializes blocks — the fix must remove true waits, not instruction
count. Next step: profile what semaphore the GpSimd sequencer polls
during the 33us gaps (GpSimd-sequencer EVENT_SEMAPHORE slices).
"""

import numpy as np
import ml_dtypes

import concourse.bass as bass
import concourse.bacc as bacc
import concourse.mybir as mybir
import concourse.tile as tile
from concourse.tile_rust import add_dep_helper

BF16 = ml_dtypes.bfloat16
F = 128
TILE = 128
WIN = 32
NEG_SLOPE = 0.2

# Full-problem geometry (hardcoded per the task contract).
N_SRC = 50000
N_DST = 10000
N_CORES = 8
HALF = 25088          # src half size (<=32768, 128-aligned)
N_SRC_PAD = 2 * HALF  # 50048
DPC = N_DST // N_CORES          # 1250 dst nodes per core
DSTPAD = ((DPC + 127) // 128) * 128   # 1280
NSB = DSTPAD // 128   # 10 superblocks
NW = DSTPAD // WIN    # 80 windows
SG = 32               # tiles per S'-build group
import os as _os
CHUNK_E = int(_os.environ.get("V1_CHUNK_E", "1024"))  # edges per dma_gather
_SCRATCH = int(_os.environ.get("V1_SCRATCH", "32768"))
ELCH = 4              # el ap_gather column chunks


# --------------------------------------------------------------------------
# Host preprocessing
# --------------------------------------------------------------------------

def _plan_core_phase(e_src, e_dst_local, half, nw, tpw):
    """Lay out one core/phase edge stream into nw*tpw tiles of 128 edges.

    e_src: local src idx (already offset by phase*half), e_dst_local: dst
    within core (sorted). Returns (g_idx, rel16, er_idx) of length
    nw*tpw*128 (int32; pads: g_idx=0, rel16=-1, er_idx=valid dst).
    """
    n_slots = nw * tpw * TILE
    g_idx = np.zeros(n_slots, np.int32)
    rel16 = np.full(n_slots, -1, np.int32)
    er_idx = np.zeros(n_slots, np.int32)

    dpc16 = nw * WIN
    cnt = np.bincount(e_dst_local, minlength=dpc16)          # per-dst counts
    pcnt = ((cnt + 15) // 16) * 16                            # padded to 16
    win_of_dst = np.arange(dpc16) // WIN
    # offset of each dst's run inside its window
    pc_cum = np.cumsum(pcnt)
    win_start_dst = np.arange(nw) * WIN
    win_pcnt_start = np.concatenate([[0], pc_cum])[win_start_dst]
    dst_off_in_win = np.concatenate([[0], pc_cum])[:-1] - win_pcnt_start[win_of_dst]
    win_tot = np.add.reduceat(pcnt, win_start_dst)
    assert (win_tot <= tpw * TILE).all(), "TPW too small for a window"
    win_base = np.arange(nw) * tpw * TILE

    # real edge positions
    e_cnt_start = np.concatenate([[0], np.cumsum(cnt)])[:-1]
    within = np.arange(len(e_src)) - np.repeat(e_cnt_start[cnt > 0],
                                               cnt[cnt > 0])
    pos = (win_base[win_of_dst[e_dst_local]]
           + dst_off_in_win[e_dst_local] + within)
    g_idx[pos] = e_src
    rel16[pos] = e_dst_local % WIN
    er_idx[pos] = e_dst_local
    # per-dst pad slots: er must stay constant within each aligned 16-run
    pad_n = pcnt - cnt
    pads = pad_n > 0
    pad_dsts = np.nonzero(pads)[0]
    if len(pad_dsts):
        pad_start = (win_base[win_of_dst[pad_dsts]]
                     + dst_off_in_win[pad_dsts] + cnt[pad_dsts])
        within_p = (np.arange(int(pad_n[pads].sum()))
                    - np.repeat(np.concatenate([[0], np.cumsum(pad_n[pads])])[:-1],
                                pad_n[pads]))
        ppos = np.repeat(pad_start, pad_n[pads]) + within_p
        er_idx[ppos] = np.repeat(pad_dsts, pad_n[pads])
    # window tail pads: er = first dst of window (any valid value)
    for w in range(nw):
        a = win_base[w] + win_tot[w]
        b = win_base[w] + tpw * TILE
        if a < b:
            er_idx[a:b] = w * WIN
    return g_idx, rel16, er_idx


def _wrap16(stream):
    """[n] -> [128, n/16] int16: element k at (k%16, k//16), tiled to 128."""
    a = stream.reshape(-1, 16).T.astype(np.int16)         # [16, n/16]
    return np.tile(a, (8, 1))


def _pack_group_streams(arr_t128, q):
    """arr_t128: [T,128] per-edge values. Build the [128, T/16] int16 idx
    tensor for ap_gather call q: group m's stream i = arr[i, 16*m+q]."""
    T = arr_t128.shape[0]
    M = arr_t128[:, q::16]                 # [T, 8]
    M2 = M.reshape(T // 16, 16, 8)         # [c, r, m]
    t = M2.transpose(2, 1, 0)              # [m, r, c]
    return np.ascontiguousarray(t.reshape(128, T // 16)).astype(np.int16)


def build_plans(src_idx, dst_idx, n_cores=N_CORES, dpc=DPC, half=HALF,
                nw=NW):
    """Returns (tpw, plans). plans[core][phase] holds the int32 layout
    arrays; tpw is the global uniform tiles-per-window."""
    src_idx = np.asarray(src_idx).astype(np.int64)
    dst_idx = np.asarray(dst_idx).astype(np.int64)
    bounds = np.searchsorted(dst_idx, np.arange(n_cores + 1) * dpc)
    split = []
    tpw = 1
    for c in range(n_cores):
        es = src_idx[bounds[c]:bounds[c + 1]].astype(np.int32)
        ed = (dst_idx[bounds[c]:bounds[c + 1]] - c * dpc).astype(np.int32)
        for ph in range(2):
            m = (es < half) if ph == 0 else (es >= half)
            ps, pd = es[m] - ph * half, ed[m]
            cnt = np.bincount(pd, minlength=nw * WIN)
            pcnt = ((cnt + 15) // 16) * 16
            wt = np.add.reduceat(pcnt, np.arange(nw) * WIN)
            tpw = max(tpw, int((wt.max() + TILE - 1) // TILE))
            split.append((c, ph, ps, pd))
    plans = [[None, None] for _ in range(n_cores)]
    for c, ph, ps, pd in split:
        plans[c][ph] = _plan_core_phase(ps, pd, half, nw, tpw)
    return tpw, plans


def build_core_inputs(plans_core, tpw, nw=NW):
    """Per-core device input arrays from the two phase plans."""
    NT = nw * tpw
    T = ((NT + 63) // 64) * 64
    out = {}
    gidx = np.zeros((2, 128, 8 * T), np.int16)
    elidx = np.zeros((2, 16, 128, T // 16), np.int16)
    eridx = np.zeros((2, 128, T // 16), np.int16)
    rel16 = np.full((2, 128, T), -1, np.float32)
    for ph in range(2):
        g, r, e = plans_core[ph]
        gT = np.zeros(T * TILE, np.int32); gT[:NT * TILE] = g
        rT = np.full(T * TILE, -1, np.int32); rT[:NT * TILE] = r
        eT = np.zeros(T * TILE, np.int32); eT[:NT * TILE] = e
        gidx[ph] = _wrap16(gT)
        a = gT.reshape(T, 128)
        ea = eT.reshape(T, 128)
        for q in range(16):
            elidx[ph, q] = _pack_group_streams(a, q)
        eridx[ph] = _pack_group_streams(ea, 0)
        rel16[ph] = rT.reshape(T, 128).T
    out["gidx"] = gidx
    out["elidx"] = elidx
    out["eridx"] = eridx
    out["rel16"] = rel16.astype(BF16)
    return out, NT, T


# --------------------------------------------------------------------------
# Bass program
# --------------------------------------------------------------------------

def build_program(tpw, nt, t_s, n_src_pad=N_SRC_PAD, half=HALF,
                  dstpad=DSTPAD, nsb=NSB, nw=NW, stage=99):
    f32 = mybir.dt.float32
    bf16 = mybir.dt.bfloat16
    i16 = mybir.dt.int16
    T = t_s
    Tc = T // 16

    nc = bacc.Bacc(None, target_bir_lowering=False,
                   dynamic_dma_scratch_size=_SCRATCH, num_swdge_queues=4)
    pool_order = [None]
    import os
    _no_chain = os.environ.get("NO_POOL_CHAIN", "0") == "1"

    def pool_op(inst):
        if not _no_chain and pool_order[0] is not None:
            add_dep_helper(inst.ins, pool_order[0].ins, False,
                           "pool library ordering")
        pool_order[0] = inst
        return inst
    x32 = nc.declare_dram_parameter("x32", [n_src_pad, F], f32, isOutput=False)
    xbf = nc.declare_dram_parameter("xbf", [n_src_pad, F], bf16, isOutput=False)
    xd = nc.declare_dram_parameter("xd", [dstpad, F], f32, isOutput=False)
    attnl = nc.declare_dram_parameter("attnl", [128, F], f32, isOutput=False)
    attnr = nc.declare_dram_parameter("attnr", [128, F], f32, isOutput=False)
    iota = nc.declare_dram_parameter("iota", [128, WIN], bf16, isOutput=False)
    pmask = nc.declare_dram_parameter("pmask", [128, 16], f32, isOutput=False)
    gidx = nc.declare_dram_parameter("gidx", [2, 128, 8 * T], i16, isOutput=False)
    elidx = nc.declare_dram_parameter("elidx", [2, 16, 128, Tc], i16, isOutput=False)
    eridx = nc.declare_dram_parameter("eridx", [2, 128, Tc], i16, isOutput=False)
    rel16 = nc.declare_dram_parameter("rel16", [2, 128, T], bf16, isOutput=False)
    outp = nc.declare_dram_parameter("out", [dstpad, F], f32, isOutput=True)
    el_d = nc.dram_tensor("el_d", [1, n_src_pad], f32)
    er_d = nc.dram_tensor("er_d", [1, dstpad], f32)

    nt_el = n_src_pad // 128
    nt_er = dstpad // 128

    with tile.TileContext(nc) as tc:
        with (
            tc.tile_pool(name="const", bufs=1) as cpool,
            tc.tile_pool(name="big", bufs=1) as bigpool,
            tc.tile_pool(name="ph", bufs=1) as phpool,
            tc.tile_pool(name="work", bufs=2) as wpool,
            tc.tile_pool(name="elw", bufs=2) as elwpool,
            tc.tile_pool(name="rows", bufs=(2 if CHUNK_E > 1024 else 4)) as rpool,
            tc.tile_pool(name="sp", bufs=3) as sppool,
            tc.tile_pool(name="psum", bufs=4, space="PSUM") as ppool,
        ):
            # ---- constants ----
            attnl_sb = cpool.tile([128, F], f32)
            nc.sync.dma_start(out=attnl_sb[:], in_=attnl[:, :])
            attnr_sb = cpool.tile([128, F], f32)
            nc.sync.dma_start(out=attnr_sb[:], in_=attnr[:, :])
            iota_sb = cpool.tile([128, WIN], bf16)
            nc.sync.dma_start(out=iota_sb[:], in_=iota[:, :])
            pmask_sb = cpool.tile([128, 16], f32)
            nc.sync.dma_start(out=pmask_sb[:], in_=pmask[:, :])
            ones_sb = cpool.tile([128, 1], bf16)
            nc.vector.memset(ones_sb[:], 1.0)
            out_acc = cpool.tile([128, nsb * (F + 1)], f32)
            nc.vector.memset(out_acc[:], 1.0)
            junk = cpool.tile([128, F], f32)

            # ---- node logits el (all src rows), er (core's dst rows) ----
            el_sb = cpool.tile([128, nt_el], f32)
            x32r2 = x32[:, :].rearrange("(t p) f -> p t f", p=128)
            ELC = 16
            for t0 in range(0, nt_el, ELC):
                ntc = min(ELC, nt_el - t0)
                xt = wpool.tile([128, ELC, F], f32, tag="xtb")
                nc.sync.dma_start(out=xt[:, :ntc, :],
                                  in_=x32r2[:, t0:t0 + ntc, :])
                for i in range(ntc):
                    nc.vector.scalar_tensor_tensor(
                        out=junk[:], in0=xt[:, i, :], scalar=1.0,
                        in1=attnl_sb[:],
                        op0=mybir.AluOpType.mult, op1=mybir.AluOpType.mult,
                        accum_out=el_sb[:, t0 + i:t0 + i + 1])
            nc.sync.dma_start(
                out=el_d[0, :].rearrange("(t p) -> p t", p=128), in_=el_sb[:])

            er_sb = cpool.tile([128, nt_er], f32)
            xdr2 = xd[:, :].rearrange("(t p) f -> p t f", p=128)
            xte = wpool.tile([128, nt_er, F], f32, tag="xte")
            nc.sync.dma_start(out=xte[:], in_=xdr2[:, :, :])
            for t in range(nt_er):
                nc.vector.scalar_tensor_tensor(
                    out=junk[:], in0=xte[:, t, :], scalar=1.0,
                    in1=attnr_sb[:],
                    op0=mybir.AluOpType.mult, op1=mybir.AluOpType.mult,
                    accum_out=er_sb[:, t:t + 1])
            nc.sync.dma_start(
                out=er_d[0, :].rearrange("(t p) -> p t", p=128), in_=er_sb[:])

            er_rep = cpool.tile([128, dstpad], f32)
            nc.sync.dma_start(out=er_rep[:],
                              in_=er_d[:, :].to_broadcast([128, dstpad]))

            # ---- phases ----
            for ph in range(2 if stage >= 1 else 0):
                el_rep = bigpool.tile([128, half], f32, tag="el_rep")
                nc.sync.dma_start(
                    out=el_rep[:],
                    in_=el_d[:, ph * half:(ph + 1) * half].to_broadcast(
                        [128, half]))

                # er per edge (single ap_gather: 16-runs share a dst)
                eridx_sb = phpool.tile([128, Tc], i16, tag="eridx")
                nc.sync.dma_start(out=eridx_sb[:], in_=eridx[ph])
                er_edge = phpool.tile([128, T], f32, tag="er_edge")
                pool_op(nc.gpsimd.ap_gather(
                    er_edge[:], er_rep[:], eridx_sb[:],
                    channels=128, num_elems=dstpad, d=1, num_idxs=T))

                # el per edge: 16 ap_gather calls (one per partition
                # residue class); assemble z = sum_q elq*mask_q (+er).
                ex_f = phpool.tile([128, T], f32, tag="ex_f")
                exb = phpool.tile([128, T], bf16, tag="exb")
                for q in range(16):
                    eq_idx = elwpool.tile([128, Tc], i16, tag="eqidx")
                    nc.sync.dma_start(out=eq_idx[:], in_=elidx[ph, q])
                    elq = elwpool.tile([128, T], f32, tag="elq")
                    pool_op(nc.gpsimd.ap_gather(
                        elq[:], el_rep[:], eq_idx[:],
                        channels=128, num_elems=half, d=1, num_idxs=T))
                    nc.vector.scalar_tensor_tensor(
                        out=ex_f[:], in0=elq[:],
                        scalar=pmask_sb[:, q:q + 1],
                        in1=(er_edge[:] if q == 0 else ex_f[:]),
                        op0=mybir.AluOpType.mult, op1=mybir.AluOpType.add)
                nc.vector.scalar_tensor_tensor(
                    out=ex_f[:], in0=ex_f[:], scalar=NEG_SLOPE, in1=ex_f[:],
                    op0=mybir.AluOpType.mult, op1=mybir.AluOpType.max)
                nc.scalar.activation(
                    out=ex_f[:], in_=ex_f[:],
                    func=mybir.ActivationFunctionType.Exp)
                nc.vector.tensor_copy(out=exb[:], in_=ex_f[:])

                rel_sb = phpool.tile([128, T], bf16, tag="rel_sb")
                nc.sync.dma_start(out=rel_sb[:], in_=rel16[ph])
                if stage < 2:
                    continue

                # ---- main tile loop ----
                n_chunks = (nt * TILE + CHUNK_E - 1) // CHUNK_E
                rows_t = None
                sp_t = None
                psum_t = None
                for t in range(nt):
                    ch, blk = divmod(t * TILE, CHUNK_E)
                    blk //= TILE
                    if stage == 2 and (ph > 0 or ch > 0):
                        continue
                    ib, ibc = divmod(ch, 16)
                    if blk == 0 and ibc == 0:
                        nei = min(16 * CHUNK_E, nt * TILE - ib * 16 * CHUNK_E)
                        gi_blk = wpool.tile([128, 16 * CHUNK_E // 16], i16,
                                            tag="gi")
                        nc.sync.dma_start(
                            out=gi_blk[:, :nei // 16],
                            in_=gidx[ph, :,
                                     ib * CHUNK_E:
                                     ib * CHUNK_E + nei // 16])
                    if blk == 0:
                        ne = min(CHUNK_E, nt * TILE - ch * CHUNK_E)
                        rows_t = rpool.tile([128, CHUNK_E // TILE, F], bf16,
                                            tag="rows")
                        pool_op(nc.gpsimd.dma_gather(
                            out_ap=rows_t[:, :ne // TILE, :],
                            in_ap=xbf[ph * half:(ph + 1) * half, :],
                            idxs_ap=gi_blk[:, ibc * (CHUNK_E // 16):
                                           ibc * (CHUNK_E // 16) + ne // 16],
                            num_idxs=ne, num_idxs_reg=ne, elem_size=F,
                            queue_num=ch % 4))
                    if stage < 3:
                        continue
                    if stage == 3 and ch > 0:
                        continue
                    if stage < 3:
                        continue
                    g, gblk = divmod(t, SG)
                    if gblk == 0:
                        sp_t = sppool.tile([128, SG * WIN], bf16, tag="sp")
                        gsl = slice(g * SG, min((g + 1) * SG, T))
                        nsg = gsl.stop - gsl.start
                        # S = (iota == rel) ; S' = S * ex
                        nc.vector.tensor_tensor(
                            out=sp_t[:, :nsg * WIN].rearrange(
                                "p (w o) -> p w o", o=WIN),
                            in0=iota_sb[:].rearrange(
                                "p (o w) -> p o w", o=1).to_broadcast(
                                    [128, nsg, WIN]),
                            in1=rel_sb[:, gsl].rearrange(
                                "p (w o) -> p w o", o=1).to_broadcast(
                                    [128, nsg, WIN]),
                            op=mybir.AluOpType.is_equal)
                        nc.vector.tensor_tensor(
                            out=sp_t[:, :nsg * WIN].rearrange(
                                "p (w o) -> p w o", o=WIN),
                            in0=sp_t[:, :nsg * WIN].rearrange(
                                "p (w o) -> p w o", o=WIN),
                            in1=exb[:, gsl].rearrange(
                                "p (w o) -> p w o", o=1).to_broadcast(
                                    [128, nsg, WIN]),
                            op=mybir.AluOpType.mult)
                    if stage < 4:
                        continue
                    w, wt = divmod(t, tpw)
                    sb, wsb = divmod(w, 128 // WIN)
                    if wsb == 0 and wt == 0:
                        psum_t = ppool.tile([128, F + 1], f32, tag="acc")
                    bp = WIN * wsb
                    first = wt == 0
                    last = wt == tpw - 1
                    lhs = sp_t[:, gblk * WIN:(gblk + 1) * WIN]
                    nc.tensor.matmul(
                        out=psum_t[bp:bp + WIN, 0:F], lhsT=lhs,
                        rhs=rows_t[:, blk, :], start=first, stop=False,
                        tile_position=(0, bp))
                    nc.tensor.matmul(
                        out=psum_t[bp:bp + WIN, F:F + 1], lhsT=lhs,
                        rhs=ones_sb[:], start=False, stop=last,
                        tile_position=(0, bp))
                    if wsb == (128 // WIN) - 1 and last:
                        osl = out_acc[:, sb * (F + 1):(sb + 1) * (F + 1)]
                        if ph == 0:
                            nc.vector.tensor_copy(out=osl, in_=psum_t[:])
                        else:
                            nc.vector.tensor_tensor(
                                out=osl, in0=osl, in1=psum_t[:],
                                op=mybir.AluOpType.add)

            # ---- normalize and write out ----
            oa3 = out_acc[:].rearrange("p (s c) -> p s c", c=F + 1)
            sx = cpool.tile([128, nsb], f32)
            nc.vector.tensor_scalar_max(out=sx[:], in0=oa3[:, :, F],
                                        scalar1=1e-30)
            rs = cpool.tile([128, nsb], f32)
            nc.vector.reciprocal(out=rs[:], in_=sx[:])
            for sb in range(nsb):
                nc.vector.tensor_scalar_mul(
                    out=oa3[:, sb, 0:F], in0=oa3[:, sb, 0:F],
                    scalar1=rs[:, sb:sb + 1])
            nc.sync.dma_start(
                out=outp[:, :].rearrange("(s p) f -> p s f", p=128),
                in_=oa3[:, :, 0:F])
    nc.finalize()
    return nc


# --------------------------------------------------------------------------
# Entry point
# --------------------------------------------------------------------------

def _host_prep(x_src, x_dst, attn_l, attn_r, src_idx, dst_idx):
    x_src = np.asarray(x_src, np.float32).reshape(-1, F)
    x_dst = np.asarray(x_dst, np.float32).reshape(-1, F)
    al = np.asarray(attn_l, np.float32).reshape(F)
    ar = np.asarray(attn_r, np.float32).reshape(F)

    tpw, plans = build_plans(src_idx, dst_idx)
    x32p = np.zeros((N_SRC_PAD, F), np.float32)
    x32p[:N_SRC] = x_src
    xbfp = x32p.astype(BF16)
    attnl_bc = np.ascontiguousarray(np.broadcast_to(al, (128, F)))
    attnr_bc = np.ascontiguousarray(np.broadcast_to(ar, (128, F)))
    iota_np = np.broadcast_to(np.arange(WIN, dtype=np.float32),
                              (128, WIN)).astype(BF16)
    iota_np = np.ascontiguousarray(iota_np)
    pmask_np = np.zeros((128, 16), np.float32)
    pmask_np[np.arange(128), np.arange(128) % 16] = 1.0

    in_maps = []
    nt = t_s = None
    for c in range(N_CORES):
        per, nt, t_s = build_core_inputs(plans[c], tpw)
        xdp = np.zeros((DSTPAD, F), np.float32)
        xdp[:DPC] = x_dst[c * DPC:(c + 1) * DPC]
        in_maps.append({
            "x32": x32p, "xbf": xbfp, "xd": xdp,
            "attnl": attnl_bc, "attnr": attnr_bc, "iota": iota_np,
            "pmask": pmask_np,
            **per,
        })
    return tpw, nt, t_s, in_maps


_CACHE = {}


def _run(x_src, x_dst, attn_l, attn_r, src_idx, dst_idx, trace=False):
    from concourse.bass_utils import run_bass_kernel_spmd
    tpw, nt, t_s, in_maps = _host_prep(x_src, x_dst, attn_l, attn_r,
                                       src_idx, dst_idx)
    key = (tpw, nt, t_s)
    if key not in _CACHE:
        _CACHE[key] = build_program(tpw, nt, t_s)
    nc = _CACHE[key]
    res = run_bass_kernel_spmd(nc, in_maps, list(range(N_CORES)),
                               trace=trace)
    outs = [np.asarray(res.results[c]["out"])[:DPC] for c in range(N_CORES)]
    out = np.concatenate(outs, axis=0).reshape(N_DST, 1, F)
    return out.astype(np.float32), res


def kernel(x_src, x_dst, attn_l, attn_r, src_idx, dst_idx):
    out, _ = _run(x_src, x_dst, attn_l, attn_r, src_idx, dst_idx)
    return out


# --------------------------------------------------------------------------
# v2: no ap_gather, el packed into gathered rows, er via PE window matmuls
# --------------------------------------------------------------------------

def _plan_core_phase_v2(e_src, e_dst_local, nw, tpw):
    """Edge stream: per window, edges (dst-sorted) padded to tpw*TILE slots.
    Pads: g_idx=0, rel=-1. No per-dst padding."""
    n_slots = nw * tpw * TILE
    g_idx = np.zeros(n_slots, np.int32)
    rel = np.full(n_slots, -1, np.int32)
    cnt = np.bincount(e_dst_local, minlength=nw * WIN)
    wtot = np.add.reduceat(cnt, np.arange(nw) * WIN)
    assert (wtot <= tpw * TILE).all()
    win_base = np.arange(nw) * tpw * TILE
    e_cnt_start = np.concatenate([[0], np.cumsum(cnt)])[:-1]
    within_all = np.arange(len(e_src)) - np.repeat(
        np.concatenate([[0], np.cumsum(wtot)])[:-1], wtot)
    w_of_e = win_of = e_dst_local // WIN
    pos = win_base[win_of] + within_all
    g_idx[pos] = e_src
    rel[pos] = e_dst_local % WIN
    return g_idx, rel


def build_plans_v2(src_idx, dst_idx, n_cores=N_CORES, dpc=DPC, half=HALF,
                   nw=NW):
    src_idx = np.asarray(src_idx).astype(np.int64)
    dst_idx = np.asarray(dst_idx).astype(np.int64)
    bounds = np.searchsorted(dst_idx, np.arange(n_cores + 1) * dpc)
    split = []
    tpw = 1
    for c in range(n_cores):
        es = src_idx[bounds[c]:bounds[c + 1]].astype(np.int32)
        ed = (dst_idx[bounds[c]:bounds[c + 1]] - c * dpc).astype(np.int32)
        for ph in range(2):
            m = (es < half) if ph == 0 else (es >= half)
            ps, pd = es[m] - ph * half, ed[m]
            cnt = np.bincount(pd, minlength=nw * WIN)
            wt = np.add.reduceat(cnt, np.arange(nw) * WIN)
            tpw = max(tpw, int((wt.max() + TILE - 1) // TILE))
            split.append((c, ph, ps, pd))
    tpw = ((tpw + 3) // 4) * 4      # nt = nw*tpw must divide by SGT=32
    plans = [[None, None] for _ in range(n_cores)]
    for c, ph, ps, pd in split:
        plans[c][ph] = _plan_core_phase_v2(ps, pd, nw, tpw)
    return tpw, plans


def build_core_inputs_v2(plans_core, tpw, nw=NW):
    NT = nw * tpw
    T = NT
    gidx = np.zeros((2, 128, 8 * T), np.int16)
    rel16 = np.full((2, 128, T), -1, np.float32)
    srt = np.zeros((2, T, 32, TILE), np.float32)
    for ph in range(2):
        g, r = plans_core[ph]
        gidx[ph] = _wrap16(g)
        rT = r.reshape(T, TILE)
        rel16[ph] = rT.T
        srt[ph] = (np.arange(WIN)[None, :, None] == rT[:, None, :])
    return {
        "gidx": gidx,
        "rel16": rel16.astype(BF16),
        "srt": srt.astype(BF16),
    }, NT, T


CH2 = 2048   # edges per gather (v2)


def build_program_v2(tpw, nt, n_src_pad=N_SRC_PAD, half=HALF,
                     dstpad=DSTPAD, nsb=NSB, nw=NW):
    f32 = mybir.dt.float32
    bf16 = mybir.dt.bfloat16
    i16 = mybir.dt.int16
    T = nt
    XL = 256                  # XE row length in bf16 elems (512B)

    nc = bacc.Bacc(None, target_bir_lowering=False,
                   dynamic_dma_scratch_size=65536)
    x32 = nc.declare_dram_parameter("x32", [n_src_pad, F], f32, isOutput=False)
    xbf = nc.declare_dram_parameter("xbf", [n_src_pad, F], bf16, isOutput=False)
    xd = nc.declare_dram_parameter("xd", [dstpad, F], f32, isOutput=False)
    attnl = nc.declare_dram_parameter("attnl", [128, F], f32, isOutput=False)
    attnr = nc.declare_dram_parameter("attnr", [128, F], f32, isOutput=False)
    iota = nc.declare_dram_parameter("iota", [128, WIN], bf16, isOutput=False)
    gidx = nc.declare_dram_parameter("gidx", [2, 128, 8 * T], i16, isOutput=False)
    rel16 = nc.declare_dram_parameter("rel16", [2, 128, T], bf16, isOutput=False)
    srt = nc.declare_dram_parameter("srt", [2, T, 32, TILE], bf16, isOutput=False)
    outp = nc.declare_dram_parameter("out", [dstpad, F], f32, isOutput=True)
    el_d = nc.dram_tensor("el_d", [1, n_src_pad], f32)
    er_d = nc.dram_tensor("er_d", [1, dstpad], f32)
    xe = nc.dram_tensor("xe", [n_src_pad, XL], bf16)

    nt_el = n_src_pad // 128
    nt_er = dstpad // 128
    WPS = 128 // WIN          # windows per superblock
    SGT = 32 if nt % 32 == 0 else 16    # tiles per S'/er group

    with tile.TileContext(nc) as tc:
        with (
            tc.tile_pool(name="const", bufs=1) as cpool,
            tc.tile_pool(name="ph", bufs=2) as phpool,
            tc.tile_pool(name="work", bufs=2) as wpool,
            tc.tile_pool(name="rows", bufs=3) as rpool,
            tc.tile_pool(name="sp", bufs=2) as sppool,
            tc.tile_pool(name="srt", bufs=2) as srtpool,
            tc.tile_pool(name="psum", bufs=2, space="PSUM") as ppool,
            tc.tile_pool(name="erps", bufs=2, space="PSUM") as epool,
        ):
            # ---- constants ----
            attnl_sb = cpool.tile([128, F], f32)
            nc.sync.dma_start(out=attnl_sb[:], in_=attnl[:, :])
            attnr_sb = cpool.tile([128, F], f32)
            nc.sync.dma_start(out=attnr_sb[:], in_=attnr[:, :])
            iota_sb = cpool.tile([128, WIN], bf16)
            nc.sync.dma_start(out=iota_sb[:], in_=iota[:, :])
            ones_sb = cpool.tile([128, 1], bf16)
            nc.vector.memset(ones_sb[:], 1.0)
            out_acc = cpool.tile([128, nsb * (F + 1)], f32)
            junk = cpool.tile([128, F], f32)

            # ---- er ----
            er_sb = cpool.tile([128, nt_er], f32)
            xdr = xd[:, :].rearrange("(t p) f -> t p f", p=128)
            for t in range(nt_er):
                xt = wpool.tile([128, F], f32, tag="xt")
                nc.sync.dma_start(out=xt[:], in_=xdr[t])
                nc.vector.scalar_tensor_tensor(
                    out=junk[:], in0=xt[:], scalar=1.0, in1=attnr_sb[:],
                    op0=mybir.AluOpType.mult, op1=mybir.AluOpType.mult,
                    accum_out=er_sb[:, t:t + 1])
            nc.sync.dma_start(
                out=er_d[0, :].rearrange("(t p) -> p t", p=128), in_=er_sb[:])
            # window-major er [32, nw] + hi/lo bf16 split
            erw = cpool.tile([32, nw], f32)
            nc.sync.dma_start(
                out=erw[:], in_=er_d[0, :].rearrange("(w r) -> r w", r=32))
            erw_hi = cpool.tile([32, nw], bf16)
            nc.vector.tensor_copy(out=erw_hi[:], in_=erw[:])
            erw_hif = cpool.tile([32, nw], f32)
            nc.vector.tensor_copy(out=erw_hif[:], in_=erw_hi[:])
            erw_lof = cpool.tile([32, nw], f32)
            nc.vector.tensor_tensor(out=erw_lof[:], in0=erw[:],
                                    in1=erw_hif[:],
                                    op=mybir.AluOpType.subtract)
            erw_lo = cpool.tile([32, nw], bf16)
            nc.vector.tensor_copy(out=erw_lo[:], in_=erw_lof[:])
            erw_hl = cpool.tile([32, 2 * nw], bf16)
            nc.vector.tensor_copy(
                out=erw_hl[:].rearrange("p (w c) -> p w c", c=2)[:, :, 0:1],
                in_=erw_hi[:].rearrange("p (w c) -> p w c", c=1))
            nc.vector.tensor_copy(
                out=erw_hl[:].rearrange("p (w c) -> p w c", c=2)[:, :, 1:2],
                in_=erw_lo[:].rearrange("p (w c) -> p w c", c=1))


            # ---- el + XE build (per src half) ----
            # (XE pad bytes stay uninitialized: gathered but never consumed)
            el_sb = cpool.tile([128, nt_el], f32)
            x32r = x32[:, :].rearrange("(t p) f -> t p f", p=128)
            xe_f32 = xe[:, :].bitcast(f32)      # [n_src_pad, 128] f32 view
            for hh in range(2):
                for tt in range(nt_el // 2):
                    t = hh * (nt_el // 2) + tt
                    xt = wpool.tile([128, F], f32, tag="xt")
                    nc.sync.dma_start(out=xt[:], in_=x32r[t])
                    nc.vector.scalar_tensor_tensor(
                        out=junk[:], in0=xt[:], scalar=1.0, in1=attnl_sb[:],
                        op0=mybir.AluOpType.mult, op1=mybir.AluOpType.mult,
                        accum_out=el_sb[:, t:t + 1])
                # bf16 x into XE cols 0:128
                h0 = hh * (n_src_pad // 2)
                nc.sync.dma_start(
                    out=xe[h0:h0 + n_src_pad // 2, 0:F],
                    in_=xbf[h0:h0 + n_src_pad // 2, :])
                # el into XE (f32 view col 64)
                nc.sync.dma_start(
                    out=xe_f32[h0:h0 + n_src_pad // 2, 64:65].rearrange(
                        "(t p) o -> p t o", p=128),
                    in_=el_sb[:, hh * (nt_el // 2):(hh + 1) * (nt_el // 2)]
                    .rearrange("p (t o) -> p t o", o=1))

            # ---- phases ----
            assert nt % SGT == 0 and (SGT * TILE) % CH2 == 0
            chunks_per_group = SGT * TILE // CH2
            for ph in range(2):
                rel_sb = phpool.tile([128, T], bf16, tag="rel_sb")
                nc.sync.dma_start(out=rel_sb[:], in_=rel16[ph])
                exb = phpool.tile([128, T], bf16, tag="exb")

                psum_t = None
                for g in range(nt // SGT):
                    rows_g = []
                    for cc in range(chunks_per_group):
                        ch = g * chunks_per_group + cc
                        ib, ibc = divmod(ch, 8)
                        if ibc == 0:
                            nei = min(8 * CH2, nt * TILE - ib * 8 * CH2)
                            gi_blk = wpool.tile([128, 8 * CH2 // 16], i16,
                                                tag="gi")
                            nc.sync.dma_start(
                                out=gi_blk[:, :nei // 16],
                                in_=gidx[ph, :,
                                         ib * (8 * CH2 // 16):
                                         ib * (8 * CH2 // 16) + nei // 16])
                        rows_t = rpool.tile([128, CH2 // TILE, XL], bf16,
                                            tag="rows")
                        nc.gpsimd.dma_gather(
                            out_ap=rows_t[:],
                            in_ap=xe[ph * half:(ph + 1) * half, :],
                            idxs_ap=gi_blk[:, ibc * (CH2 // 16):
                                           (ibc + 1) * (CH2 // 16)],
                            num_idxs=CH2, num_idxs_reg=CH2, elem_size=XL)
                        rows_g.append(rows_t)
                    gsl = slice(g * SGT, (g + 1) * SGT)
                    # er matmuls (uploaded one-hot transposes @ er windows)
                    srt_t = srtpool.tile([32, SGT * TILE], bf16, tag="srt")
                    nc.sync.dma_start(
                        out=srt_t[:].rearrange("p (t e) -> p t e", e=TILE),
                        in_=srt[ph, gsl].rearrange("t r e -> r t e"))
                    erps_t = epool.tile([128, 2 * SGT], f32, tag="erps")
                    for gblk in range(SGT):
                        t = g * SGT + gblk
                        nc.tensor.matmul(
                            out=erps_t[:, 2 * gblk:2 * gblk + 2],
                            lhsT=srt_t[:, gblk * TILE:(gblk + 1) * TILE],
                            rhs=erw_hl[:, 2 * (t // tpw):2 * (t // tpw) + 2],
                            start=True, stop=True)
                    # S'raw = (iota == rel)
                    sp_t = sppool.tile([128, SGT * WIN], bf16, tag="sp")
                    nc.vector.tensor_tensor(
                        out=sp_t[:].rearrange("p (w o) -> p w o", o=WIN),
                        in0=iota_sb[:].rearrange(
                            "p (o w) -> p o w", o=1).to_broadcast(
                                [128, SGT, WIN]),
                        in1=rel_sb[:, gsl].rearrange(
                            "p (w o) -> p w o", o=1).to_broadcast(
                                [128, SGT, WIN]),
                        op=mybir.AluOpType.is_equal)
                    # z = el + er_hi + er_lo ; lrelu ; exp  (batched)
                    zg = wpool.tile([128, SGT], f32, tag="zg")
                    tpc = CH2 // TILE
                    for cc in range(chunks_per_group):
                        el_col = rows_g[cc][:, :, F:F + 2].bitcast(f32)
                        nc.vector.tensor_tensor(
                            out=zg[:, cc * tpc:(cc + 1) * tpc],
                            in0=el_col.rearrange("p a b -> p (a b)"),
                            in1=erps_t[:].rearrange(
                                "p (t c) -> p t c", c=2)[
                                :, cc * tpc:(cc + 1) * tpc, 0],
                            op=mybir.AluOpType.add)
                    nc.vector.tensor_tensor(
                        out=zg[:], in0=zg[:],
                        in1=erps_t[:].rearrange(
                            "p (t c) -> p t c", c=2)[:, :, 1],
                        op=mybir.AluOpType.add)
                    nc.vector.scalar_tensor_tensor(
                        out=zg[:], in0=zg[:], scalar=NEG_SLOPE, in1=zg[:],
                        op0=mybir.AluOpType.mult, op1=mybir.AluOpType.max)
                    nc.scalar.activation(
                        out=exb[:, gsl], in_=zg[:],
                        func=mybir.ActivationFunctionType.Exp)
                    # S' = S'raw * ex
                    nc.vector.tensor_tensor(
                        out=sp_t[:].rearrange("p (w o) -> p w o", o=WIN),
                        in0=sp_t[:].rearrange("p (w o) -> p w o", o=WIN),
                        in1=exb[:, gsl].rearrange(
                            "p (w o) -> p w o", o=1).to_broadcast(
                                [128, SGT, WIN]),
                        op=mybir.AluOpType.mult)
                    # segment matmuls
                    for gblk in range(SGT):
                        t = g * SGT + gblk
                        w, wt = divmod(t, tpw)
                        sb, wsb = divmod(w, WPS)
                        if wsb == 0 and wt == 0:
                            psum_t = ppool.tile([128, F + 1], f32, tag="acc")
                        bp = WIN * wsb
                        first = wt == 0
                        last = wt == tpw - 1
                        lhs = sp_t[:, gblk * WIN:(gblk + 1) * WIN]
                        cc, blk = divmod(gblk, tpc)
                        nc.tensor.matmul(
                            out=psum_t[bp:bp + WIN, 0:F], lhsT=lhs,
                            rhs=rows_g[cc][:, blk, 0:F], start=first,
                            stop=False, tile_position=(0, bp))
                        nc.tensor.matmul(
                            out=psum_t[bp:bp + WIN, F:F + 1], lhsT=lhs,
                            rhs=ones_sb[:], start=False, stop=last,
                            tile_position=(0, bp))
                        if wsb == WPS - 1 and last:
                            osl = out_acc[:,
                                          sb * (F + 1):(sb + 1) * (F + 1)]
                            if ph == 0:
                                nc.vector.tensor_copy(out=osl,
                                                      in_=psum_t[:])
                            else:
                                nc.vector.tensor_tensor(
                                    out=osl, in0=osl, in1=psum_t[:],
                                    op=mybir.AluOpType.add)

            # ---- normalize and write out ----
            oa3 = out_acc[:].rearrange("p (s c) -> p s c", c=F + 1)
            sx = cpool.tile([128, nsb], f32)
            nc.vector.tensor_scalar_max(out=sx[:], in0=oa3[:, :, F],
                                        scalar1=1e-30)
            rs = cpool.tile([128, nsb], f32)
            nc.vector.reciprocal(out=rs[:], in_=sx[:])
            for sb in range(nsb):
                nc.vector.tensor_scalar_mul(
                    out=oa3[:, sb, 0:F], in0=oa3[:, sb, 0:F],
                    scalar1=rs[:, sb:sb + 1])
            nc.sync.dma_start(
                out=outp[:, :].rearrange("(s p) f -> p s f", p=128),
                in_=oa3[:, :, 0:F])
    nc.finalize()
    return nc


def _host_prep_v2(x_src, x_dst, attn_l, attn_r, src_idx, dst_idx):
    x_src = np.asarray(x_src, np.float32).reshape(-1, F)
    x_dst = np.asarray(x_dst, np.float32).reshape(-1, F)
    al = np.asarray(attn_l, np.float32).reshape(F)
    ar = np.asarray(attn_r, np.float32).reshape(F)
    tpw, plans = build_plans_v2(src_idx, dst_idx)
    x32p = np.zeros((N_SRC_PAD, F), np.float32)
    x32p[:N_SRC] = x_src
    xbfp = x32p.astype(BF16)
    attnl_bc = np.ascontiguousarray(np.broadcast_to(al, (128, F)))
    attnr_bc = np.ascontiguousarray(np.broadcast_to(ar, (128, F)))
    iota_np = np.ascontiguousarray(
        np.broadcast_to(np.arange(WIN, dtype=np.float32),
                        (128, WIN))).astype(BF16)
    in_maps = []
    nt = None
    for c in range(N_CORES):
        per, nt, _ = build_core_inputs_v2(plans[c], tpw)
        xdp = np.zeros((DSTPAD, F), np.float32)
        xdp[:DPC] = x_dst[c * DPC:(c + 1) * DPC]
        in_maps.append({
            "x32": x32p, "xbf": xbfp, "xd": xdp,
            "attnl": attnl_bc, "attnr": attnr_bc, "iota": iota_np,
            **per,
        })
    return tpw, nt, in_maps


def _run_v2(x_src, x_dst, attn_l, attn_r, src_idx, dst_idx, trace=False):
    from concourse.bass_utils import run_bass_kernel_spmd
    tpw, nt, in_maps = _host_prep_v2(x_src, x_dst, attn_l, attn_r,
                                     src_idx, dst_idx)
    key = ("v2", tpw, nt)
    if key not in _CACHE:
        _CACHE[key] = build_program_v2(tpw, nt)
    nc = _CACHE[key]
    res = run_bass_kernel_spmd(nc, in_maps, list(range(N_CORES)),
                               trace=trace)
    outs = [np.asarray(res.results[c]["out"])[:DPC] for c in range(N_CORES)]
    out = np.concatenate(outs, axis=0).reshape(N_DST, 1, F)
    return out.astype(np.float32), res


# --------------------------------------------------------------------------
# v3: scatter-W design — no per-edge DMA. W[src%128, dst] built per 128-src
# chunk via GPSIMD local_scatter of bf16 attention values; out accumulated
# as x_c.T @ W_c on PE across 391 chunks; sum_ex from bf16 W accumulator.
# --------------------------------------------------------------------------

NCH = N_SRC_PAD // 128          # 391 src chunks
NBLK = 4                        # z-pipeline blocks


def _prep_v3(src_idx, dst_idx):
    src = np.asarray(src_idx).astype(np.int64)
    dst = np.asarray(dst_idx).astype(np.int64)
    bounds = np.searchsorted(dst, np.arange(N_CORES + 1) * DPC)
    per_core = []
    cnts = np.zeros((N_CORES, NCH * 128), np.int64)
    for c8 in range(N_CORES):
        s = src[bounds[c8]:bounds[c8 + 1]]
        v = dst[bounds[c8]:bounds[c8 + 1]] - c8 * DPC
        key = s * 2048 + v
        uk, kc = np.unique(key, return_counts=True)
        us = (uk >> 11).astype(np.int64)
        uv = (uk & 2047).astype(np.int64)
        cnts[c8] = np.bincount(us, minlength=NCH * 128)
        per_core.append((us, uv, kc))
    ncell = cnts.max(0).reshape(NCH, 128).max(1)
    n_c = np.maximum(((ncell + 1) // 2) * 2, 2).astype(np.int64)
    bl = [0, 98, 196, 294, NCH]
    off = np.zeros(NCH, np.int64)
    col_blocks = []
    cur = 0
    for b in range(NBLK):
        cs = cur
        for c in range(bl[b], bl[b + 1]):
            off[c] = cur
            cur += n_c[c]
        cur = ((cur + 15) // 16) * 16
        col_blocks.append((cs, cur))
    T = cur

    ch_of_col = np.zeros(T, np.int64)
    for c in range(NCH):
        ch_of_col[off[c]:off[c] + n_c[c]] = c
    a = ch_of_col.reshape(T // 16, 16).T.astype(np.int16)
    elidx = np.tile(a, (8, 1))

    streams = []
    for c8 in range(N_CORES):
        us, uv, kc = per_core[c8]
        order = np.argsort(us, kind="stable")
        us_o, uv_o, kc_o = us[order], uv[order], kc[order]
        cellcnt = np.bincount(us_o, minlength=NCH * 128)
        starts = np.concatenate([[0], np.cumsum(cellcnt)])[:-1]
        j = np.arange(len(us_o)) - starts[us_o]
        cols = off[us_o // 128] + j
        rows = us_o % 128
        scat = np.full((128, T), -1, np.int16)
        scat[rows, cols] = uv_o.astype(np.int16)
        kfac = np.zeros((128, T), np.float32)
        kfac[rows, cols] = kc_o
        vfull = np.where(scat >= 0, scat, 0).astype(np.int16)
        eridx = np.zeros((16, 128, T // 16), np.int16)
        for q in range(16):
            sm = vfull[q::16, :]
            aa = sm.reshape(8, T // 16, 16).transpose(0, 2, 1)
            eridx[q] = aa.reshape(128, T // 16)
        streams.append({"scat": scat, "kfac": kfac.astype(BF16),
                        "eridx": eridx})
    return T, n_c, off, bl, col_blocks, elidx, streams


def build_program_v3(T, n_c, off, bl, col_blocks):
    f32 = mybir.dt.float32
    bf16 = mybir.dt.bfloat16
    i16 = mybir.dt.int16

    nc = bacc.Bacc(None, target_bir_lowering=False)
    xdev = nc.declare_dram_parameter("xdev", [128, NCH * F], bf16, isOutput=False)
    xddev = nc.declare_dram_parameter("xddev", [128, NSB * F], f32, isOutput=False)
    attnl = nc.declare_dram_parameter("attnl", [128, F], bf16, isOutput=False)
    attnr = nc.declare_dram_parameter("attnr", [128, F], f32, isOutput=False)
    pmask = nc.declare_dram_parameter("pmask", [128, 16], f32, isOutput=False)
    scat = nc.declare_dram_parameter("scat", [128, T], i16, isOutput=False)
    kfac = nc.declare_dram_parameter("kfac", [128, T], bf16, isOutput=False)
    eridx = nc.declare_dram_parameter("eridx", [16, 128, T // 16], i16, isOutput=False)
    elidx = nc.declare_dram_parameter("elidx", [128, T // 16], i16, isOutput=False)
    outp = nc.declare_dram_parameter("out", [128, DSTPAD], f32, isOutput=True)
    er_d = nc.dram_tensor("er_d", [1, DSTPAD], f32)
    sums_d = nc.dram_tensor("sums_d", [1, DSTPAD], f32)

    bTmax = max(ce - cs for cs, ce in col_blocks)

    with tile.TileContext(nc) as tc:
        with (
            tc.tile_pool(name="const", bufs=1) as cpool,
            tc.tile_pool(name="z", bufs=1) as zpool,


# revision 8
# speedup vs baseline: 1.0866x; 1.0866x over previous
"""GAT-style message passing (DistSageConv) on 8 Trainium2 NeuronCores.

Strategy (dst-sharded, single SPMD program):
  - Core c owns dst nodes [c*1250, (c+1)*1250). dst_idx is sorted, so each
    core's edges are a contiguous slice of the edge list.
  - Host preprocessing (integer index manipulation + dtype casts only):
    per core, edges are split into 2 phases by src half (keeps gather
    indices < 32768 for int16 DMA-gather), grouped by dst, padded so that
      * every dst's per-phase edge run is a multiple of 16 (lets the
        GPSIMD ap_gather 16-partition groups deliver per-edge er/el),
      * every 16-dst window owns exactly TPW 128-edge tiles (uniform
        program structure across cores + clean PSUM accumulation groups).
  - Device per core:
      el[u] = sum_f x_src[u,f]*attn_l[f]  (f32, DVE STT with accum)
      er[v] likewise from the core's x_dst rows.
      per-edge el/er delivered via GPSIMD ap_gather from partition-
      replicated tables; z = el+er; leaky_relu; exp on ACT.
      x rows gathered in bf16 via SWDGE dma_gather (256B elements).
      S'[e, w] = (rel_dst[e]==w) * ex[e]  built on DVE (bf16).
      PE matmuls accumulate in PSUM: U += S'.T @ rows ; sum_ex += S'.T @ 1.
      out = U / max(sum_ex, tiny); DMA out per core; host concatenates.

Perf notes (2026-08-08): baseline 3.49ms -> 2.33ms by batching the el
preamble x-stream into 16-tile DMAs (was 401 x ~2us fixed-cost DMAs)
and deepening rows/sp/psum pools. A v3 rewrite (below, _run_v3) builds
dense W[128src x 1280dst] bf16 tiles per 128-src chunk via GPSIMD
local_scatter and streams x_c.T @ W_c on PE (no per-edge DMA at all;
passes at rel_err 7.2e-3). Its engine-busy totals are ~600us/engine,
but wall is 3.0-3.2ms: every GPSIMD instruction that must WAIT on an
upstream dep (gq/wt buffer released by DVE/PE) stalls a constant
~33.3us (observed in traces as exact 33,340ns APGather spacing in the
z-phase, both with and without scatter interleave). To make v3 win,
restructure so GPSIMD never blocks: pre-stage all idx tiles in SBUF,
give every ap_gather a disjoint output slice (no gq buffer reuse),
and size wt pools so local_scatter stays ahead of PE consumers.
Batching 4 er-residues per ap_gather call with gq/zf bufs=1 made it
WORSE (3.65ms): fewer instructions but each still waits, and bufs=1
serializes blocks — the fix must remove true waits, not instruction
count. Next step: profile what semaphore the GpSimd sequencer polls
during the 33us gaps (GpSimd-sequencer EVENT_SEMAPHORE slices).
"""

import numpy as np
import ml_dtypes

import concourse.bass as bass
import concourse.bacc as bacc
import concourse.mybir as mybir
import concourse.tile as tile
from concourse.tile_rust import add_dep_helper

BF16 = ml_dtypes.bfloat16
F = 128
TILE = 128
WIN = 32
NEG_SLOPE = 0.2

# Full-problem geometry (hardcoded per the task contract).
N_SRC = 50000
N_DST = 10000
N_CORES = 8
HALF = 25088          # src half size (<=32768, 128-aligned)
N_SRC_PAD = 2 * HALF  # 50048
DPC = N_DST // N_CORES          # 1250 dst nodes per core
DSTPAD = ((DPC + 127) // 128) * 128   # 1280
NSB = DSTPAD // 128   # 10 superblocks
NW = DSTPAD // WIN    # 80 windows
SG = 32               # tiles per S'-build group
import os as _os
CHUNK_E = int(_os.environ.get("V1_CHUNK_E", "1024"))  # edges per dma_gather
_SCRATCH = int(_os.environ.get("V1_SCRATCH", "32768"))
ELCH = 4              # el ap_gather column chunks


# --------------------------------------------------------------------------
# Host preprocessing
# --------------------------------------------------------------------------

def _plan_core_phase(e_src, e_dst_local, half, nw, tpw):
    """Lay out one core/phase edge stream into nw*tpw tiles of 128 edges.

    e_src: local src idx (already offset by phase*half), e_dst_local: dst
    within core (sorted). Returns (g_idx, rel16, er_idx) of length
    nw*tpw*128 (int32; pads: g_idx=0, rel16=-1, er_idx=valid dst).
    """
    n_slots = nw * tpw * TILE
    g_idx = np.zeros(n_slots, np.int32)
    rel16 = np.full(n_slots, -1, np.int32)
    er_idx = np.zeros(n_slots, np.int32)

    dpc16 = nw * WIN
    cnt = np.bincount(e_dst_local, minlength=dpc16)          # per-dst counts
    pcnt = ((cnt + 15) // 16) * 16                            # padded to 16
    win_of_dst = np.arange(dpc16) // WIN
    # offset of each dst's run inside its window
    pc_cum = np.cumsum(pcnt)
    win_start_dst = np.arange(nw) * WIN
    win_pcnt_start = np.concatenate([[0], pc_cum])[win_start_dst]
    dst_off_in_win = np.concatenate([[0], pc_cum])[:-1] - win_pcnt_start[win_of_dst]
    win_tot = np.add.reduceat(pcnt, win_start_dst)
    assert (win_tot <= tpw * TILE).all(), "TPW too small for a window"
    win_base = np.arange(nw) * tpw * TILE

    # real edge positions
    e_cnt_start = np.concatenate([[0], np.cumsum(cnt)])[:-1]
    within = np.arange(len(e_src)) - np.repeat(e_cnt_start[cnt > 0],
                                               cnt[cnt > 0])
    pos = (win_base[win_of_dst[e_dst_local]]
           + dst_off_in_win[e_dst_local] + within)
    g_idx[pos] = e_src
    rel16[pos] = e_dst_local % WIN
    er_idx[pos] = e_dst_local
    # per-dst pad slots: er must stay constant within each aligned 16-run
    pad_n = pcnt - cnt
    pads = pad_n > 0
    pad_dsts = np.nonzero(pads)[0]
    if len(pad_dsts):
        pad_start = (win_base[win_of_dst[pad_dsts]]
                     + dst_off_in_win[pad_dsts] + cnt[pad_dsts])
        within_p = (np.arange(int(pad_n[pads].sum()))
                    - np.repeat(np.concatenate([[0], np.cumsum(pad_n[pads])])[:-1],
                                pad_n[pads]))
        ppos = np.repeat(pad_start, pad_n[pads]) + within_p
        er_idx[ppos] = np.repeat(pad_dsts, pad_n[pads])
    # window tail pads: er = first dst of window (any valid value)
    for w in range(nw):
        a = win_base[w] + win_tot[w]
        b = win_base[w] + tpw * TILE
        if a < b:
            er_idx[a:b] = w * WIN
    return g_idx, rel16, er_idx


def _wrap16(stream):
    """[n] -> [128, n/16] int16: element k at (k%16, k//16), tiled to 128."""
    a = stream.reshape(-1, 16).T.astype(np.int16)         # [16, n/16]
    return np.tile(a, (8, 1))


def _pack_group_streams(arr_t128, q):
    """arr_t128: [T,128] per-edge values. Build the [128, T/16] int16 idx
    tensor for ap_gather call q: group m's stream i = arr[i, 16*m+q]."""
    T = arr_t128.shape[0]
    M = arr_t128[:, q::16]                 # [T, 8]
    M2 = M.reshape(T // 16, 16, 8)         # [c, r, m]
    t = M2.transpose(2, 1, 0)              # [m, r, c]
    return np.ascontiguousarray(t.reshape(128, T // 16)).astype(np.int16)


def build_plans(src_idx, dst_idx, n_cores=N_CORES, dpc=DPC, half=HALF,
                nw=NW):
    """Returns (tpw, plans). plans[core][phase] holds the int32 layout
    arrays; tpw is the global uniform tiles-per-window."""
    src_idx = np.asarray(src_idx).astype(np.int64)
    dst_idx = np.asarray(dst_idx).astype(np.int64)
    bounds = np.searchsorted(dst_idx, np.arange(n_cores + 1) * dpc)
    split = []
    tpw = 1
    for c in range(n_cores):
        es = src_idx[bounds[c]:bounds[c + 1]].astype(np.int32)
        ed = (dst_idx[bounds[c]:bounds[c + 1]] - c * dpc).astype(np.int32)
        for ph in range(2):
            m = (es < half) if ph == 0 else (es >= half)
            ps, pd = es[m] - ph * half, ed[m]
            cnt = np.bincount(pd, minlength=nw * WIN)
            pcnt = ((cnt + 15) // 16) * 16
            wt = np.add.reduceat(pcnt, np.arange(nw) * WIN)
            tpw = max(tpw, int((wt.max() + TILE - 1) // TILE))
            split.append((c, ph, ps, pd))
    plans = [[None, None] for _ in range(n_cores)]
    for c, ph, ps, pd in split:
        plans[c][ph] = _plan_core_phase(ps, pd, half, nw, tpw)
    return tpw, plans


def build_core_inputs(plans_core, tpw, nw=NW):
    """Per-core device input arrays from the two phase plans."""
    NT = nw * tpw
    T = ((NT + 63) // 64) * 64
    out = {}
    gidx = np.zeros((2, 128, 8 * T), np.int16)
    elidx = np.zeros((2, 16, 128, T // 16), np.int16)
    eridx = np.zeros((2, 128, T // 16), np.int16)
    rel16 = np.full((2, 128, T), -1, np.float32)
    for ph in range(2):
        g, r, e = plans_core[ph]
        gT = np.zeros(T * TILE, np.int32); gT[:NT * TILE] = g
        rT = np.full(T * TILE, -1, np.int32); rT[:NT * TILE] = r
        eT = np.zeros(T * TILE, np.int32); eT[:NT * TILE] = e
        gidx[ph] = _wrap16(gT)
        a = gT.reshape(T, 128)
        ea = eT.reshape(T, 128)
        for q in range(16):
            elidx[ph, q] = _pack_group_streams(a, q)
        eridx[ph] = _pack_group_streams(ea, 0)
        rel16[ph] = rT.reshape(T, 128).T
    out["gidx"] = gidx
    out["elidx"] = elidx
    out["eridx"] = eridx
    out["rel16"] = rel16.astype(BF16)
    return out, NT, T


# --------------------------------------------------------------------------
# Bass program
# --------------------------------------------------------------------------

def build_program(tpw, nt, t_s, n_src_pad=N_SRC_PAD, half=HALF,
                  dstpad=DSTPAD, nsb=NSB, nw=NW, stage=99):
    f32 = mybir.dt.float32
    bf16 = mybir.dt.bfloat16
    i16 = mybir.dt.int16
    T = t_s
    Tc = T // 16

    nc = bacc.Bacc(None, target_bir_lowering=False,
                   dynamic_dma_scratch_size=_SCRATCH, num_swdge_queues=4)
    pool_order = [None]
    import os
    _no_chain = os.environ.get("NO_POOL_CHAIN", "0") == "1"

    def pool_op(inst):
        if not _no_chain and pool_order[0] is not None:
            add_dep_helper(inst.ins, pool_order[0].ins, False,
                           "pool library ordering")
        pool_order[0] = inst
        return inst
    x32 = nc.declare_dram_parameter("x32", [n_src_pad, F], f32, isOutput=False)
    xbf = nc.declare_dram_parameter("xbf", [n_src_pad, F], bf16, isOutput=False)
    xd = nc.declare_dram_parameter("xd", [dstpad, F], f32, isOutput=False)
    attnl = nc.declare_dram_parameter("attnl", [128, F], f32, isOutput=False)
    attnr = nc.declare_dram_parameter("attnr", [128, F], f32, isOutput=False)
    iota = nc.declare_dram_parameter("iota", [128, WIN], bf16, isOutput=False)
    pmask = nc.declare_dram_parameter("pmask", [128, 16], f32, isOutput=False)
    gidx = nc.declare_dram_parameter("gidx", [2, 128, 8 * T], i16, isOutput=False)
    elidx = nc.declare_dram_parameter("elidx", [2, 16, 128, Tc], i16, isOutput=False)
    eridx = nc.declare_dram_parameter("eridx", [2, 128, Tc], i16, isOutput=False)
    rel16 = nc.declare_dram_parameter("rel16", [2, 128, T], bf16, isOutput=False)
    outp = nc.declare_dram_parameter("out", [dstpad, F], f32, isOutput=True)
    el_d = nc.dram_tensor("el_d", [1, n_src_pad], f32)
    er_d = nc.dram_tensor("er_d", [1, dstpad], f32)

    nt_el = n_src_pad // 128
    nt_er = dstpad // 128

    with tile.TileContext(nc) as tc:
        with (
            tc.tile_pool(name="const", bufs=1) as cpool,
            tc.tile_pool(name="big", bufs=1) as bigpool,
            tc.tile_pool(name="ph", bufs=1) as phpool,
            tc.tile_pool(name="work", bufs=2) as wpool,
            tc.tile_pool(name="elw", bufs=2) as elwpool,
            tc.tile_pool(name="rows", bufs=(2 if CHUNK_E > 1024 else 4)) as rpool,
            tc.tile_pool(name="sp", bufs=3) as sppool,
            tc.tile_pool(name="psum", bufs=4, space="PSUM") as ppool,
        ):
            # ---- constants ----
            attnl_sb = cpool.tile([128, F], f32)
            nc.sync.dma_start(out=attnl_sb[:], in_=attnl[:, :])
            attnr_sb = cpool.tile([128, F], f32)
            nc.sync.dma_start(out=attnr_sb[:], in_=attnr[:, :])
            iota_sb = cpool.tile([128, WIN], bf16)
            nc.sync.dma_start(out=iota_sb[:], in_=iota[:, :])
            pmask_sb = cpool.tile([128, 16], f32)
            nc.sync.dma_start(out=pmask_sb[:], in_=pmask[:, :])
            ones_sb = cpool.tile([128, 1], bf16)
            nc.vector.memset(ones_sb[:], 1.0)
            out_acc = cpool.tile([128, nsb * (F + 1)], f32)
            nc.vector.memset(out_acc[:], 1.0)
            junk = cpool.tile([128, F], f32)

            # ---- node logits el (all src rows), er (core's dst rows) ----
            el_sb = cpool.tile([128, nt_el], f32)
            x32r2 = x32[:, :].rearrange("(t p) f -> p t f", p=128)
            ELC = 16
            for t0 in range(0, nt_el, ELC):
                ntc = min(ELC, nt_el - t0)
                xt = wpool.tile([128, ELC, F], f32, tag="xtb")
                nc.sync.dma_start(out=xt[:, :ntc, :],
                                  in_=x32r2[:, t0:t0 + ntc, :])
                for i in range(ntc):
                    nc.vector.scalar_tensor_tensor(
                        out=junk[:], in0=xt[:, i, :], scalar=1.0,
                        in1=attnl_sb[:],
                        op0=mybir.AluOpType.mult, op1=mybir.AluOpType.mult,
                        accum_out=el_sb[:, t0 + i:t0 + i + 1])
            nc.sync.dma_start(
                out=el_d[0, :].rearrange("(t p) -> p t", p=128), in_=el_sb[:])

            er_sb = cpool.tile([128, nt_er], f32)
            xdr2 = xd[:, :].rearrange("(t p) f -> p t f", p=128)
            xte = wpool.tile([128, nt_er, F], f32, tag="xte")
            nc.sync.dma_start(out=xte[:], in_=xdr2[:, :, :])
            for t in range(nt_er):
                nc.vector.scalar_tensor_tensor(
                    out=junk[:], in0=xte[:, t, :], scalar=1.0,
                    in1=attnr_sb[:],
                    op0=mybir.AluOpType.mult, op1=mybir.AluOpType.mult,
                    accum_out=er_sb[:, t:t + 1])
            nc.sync.dma_start(
                out=er_d[0, :].rearrange("(t p) -> p t", p=128), in_=er_sb[:])

            er_rep = cpool.tile([128, dstpad], f32)
            nc.sync.dma_start(out=er_rep[:],
                              in_=er_d[:, :].to_broadcast([128, dstpad]))

            # ---- phases ----
            for ph in range(2 if stage >= 1 else 0):
                el_rep = bigpool.tile([128, half], f32, tag="el_rep")
                nc.sync.dma_start(
                    out=el_rep[:],
                    in_=el_d[:, ph * half:(ph + 1) * half].to_broadcast(
                        [128, half]))

                # er per edge (single ap_gather: 16-runs share a dst)
                eridx_sb = phpool.tile([128, Tc], i16, tag="eridx")
                nc.sync.dma_start(out=eridx_sb[:], in_=eridx[ph])
                er_edge = phpool.tile([128, T], f32, tag="er_edge")
                pool_op(nc.gpsimd.ap_gather(
                    er_edge[:], er_rep[:], eridx_sb[:],
                    channels=128, num_elems=dstpad, d=1, num_idxs=T))

                # el per edge: 16 ap_gather calls (one per partition
                # residue class); assemble z = sum_q elq*mask_q (+er).
                ex_f = phpool.tile([128, T], f32, tag="ex_f")
                exb = phpool.tile([128, T], bf16, tag="exb")
                for q in range(16):
                    eq_idx = elwpool.tile([128, Tc], i16, tag="eqidx")
                    nc.sync.dma_start(out=eq_idx[:], in_=elidx[ph, q])
                    elq = elwpool.tile([128, T], f32, tag="elq")
                    pool_op(nc.gpsimd.ap_gather(
                        elq[:], el_rep[:], eq_idx[:],
                        channels=128, num_elems=half, d=1, num_idxs=T))
                    nc.vector.scalar_tensor_tensor(
                        out=ex_f[:], in0=elq[:],
                        scalar=pmask_sb[:, q:q + 1],
                        in1=(er_edge[:] if q == 0 else ex_f[:]),
                        op0=mybir.AluOpType.mult, op1=mybir.AluOpType.add)
                nc.vector.scalar_tensor_tensor(
                    out=ex_f[:], in0=ex_f[:], scalar=NEG_SLOPE, in1=ex_f[:],
                    op0=mybir.AluOpType.mult, op1=mybir.AluOpType.max)
                nc.scalar.activation(
                    out=ex_f[:], in_=ex_f[:],
                    func=mybir.ActivationFunctionType.Exp)
                nc.vector.tensor_copy(out=exb[:], in_=ex_f[:])

                rel_sb = phpool.tile([128, T], bf16, tag="rel_sb")
                nc.sync.dma_start(out=rel_sb[:], in_=rel16[ph])
                if stage < 2:
                    continue

                # ---- main tile loop ----
                n_chunks = (nt * TILE + CHUNK_E - 1) // CHUNK_E
                rows_t = None
                sp_t = None
                psum_t = None
                for t in range(nt):
                    ch, blk = divmod(t * TILE, CHUNK_E)
                    blk //= TILE
                    if stage == 2 and (ph > 0 or ch > 0):
                        continue
                    ib, ibc = divmod(ch, 16)
                    if blk == 0 and ibc == 0:
                        nei = min(16 * CHUNK_E, nt * TILE - ib * 16 * CHUNK_E)
                        gi_blk = wpool.tile([128, 16 * CHUNK_E // 16], i16,
                                            tag="gi")
                        nc.sync.dma_start(
                            out=gi_blk[:, :nei // 16],
                            in_=gidx[ph, :,
                                     ib * CHUNK_E:
                                     ib * CHUNK_E + nei // 16])
                    if blk == 0:
                        ne = min(CHUNK_E, nt * TILE - ch * CHUNK_E)
                        rows_t = rpool.tile([128, CHUNK_E // TILE, F], bf16,
                                            tag="rows")
                        pool_op(nc.gpsimd.dma_gather(
                            out_ap=rows_t[:, :ne // TILE, :],
                            in_ap=xbf[ph * half:(ph + 1) * half, :],
                            idxs_ap=gi_blk[:, ibc * (CHUNK_E // 16):
                                           ibc * (CHUNK_E // 16) + ne // 16],
                            num_idxs=ne, num_idxs_reg=ne, elem_size=F,
                            queue_num=ch % 4))
                    if stage < 3:
                        continue
                    if stage == 3 and ch > 0:
                        continue
                    if stage < 3:
                        continue
                    g, gblk = divmod(t, SG)
                    if gblk == 0:
                        sp_t = sppool.tile([128, SG * WIN], bf16, tag="sp")
                        gsl = slice(g * SG, min((g + 1) * SG, T))
                        nsg = gsl.stop - gsl.start
                        # S = (iota == rel) ; S' = S * ex
                        nc.vector.tensor_tensor(
                            out=sp_t[:, :nsg * WIN].rearrange(
                                "p (w o) -> p w o", o=WIN),
                            in0=iota_sb[:].rearrange(
                                "p (o w) -> p o w", o=1).to_broadcast(
                                    [128, nsg, WIN]),
                            in1=rel_sb[:, gsl].rearrange(
                                "p (w o) -> p w o", o=1).to_broadcast(
                                    [128, nsg, WIN]),
                            op=mybir.AluOpType.is_equal)
                        nc.vector.tensor_tensor(
                            out=sp_t[:, :nsg * WIN].rearrange(
                                "p (w o) -> p w o", o=WIN),
                            in0=sp_t[:, :nsg * WIN].rearrange(
                                "p (w o) -> p w o", o=WIN),
                            in1=exb[:, gsl].rearrange(
                                "p (w o) -> p w o", o=1).to_broadcast(
                                    [128, nsg, WIN]),
                            op=mybir.AluOpType.mult)
                    if stage < 4:
                        continue
                    w, wt = divmod(t, tpw)
                    sb, wsb = divmod(w, 128 // WIN)
                    if wsb == 0 and wt == 0:
                        psum_t = ppool.tile([128, F + 1], f32, tag="acc")
                    bp = WIN * wsb
                    first = wt == 0
                    last = wt == tpw - 1
                    lhs = sp_t[:, gblk * WIN:(gblk + 1) * WIN]
                    nc.tensor.matmul(
                        out=psum_t[bp:bp + WIN, 0:F], lhsT=lhs,
                        rhs=rows_t[:, blk, :], start=first, stop=False,
                        tile_position=(0, bp))
                    nc.tensor.matmul(
                        out=psum_t[bp:bp + WIN, F:F + 1], lhsT=lhs,
                        rhs=ones_sb[:], start=False, stop=last,
                        tile_position=(0, bp))
                    if wsb == (128 // WIN) - 1 and last:
                        osl = out_acc[:, sb * (F + 1):(sb + 1) * (F + 1)]
                        if ph == 0:
                            nc.vector.tensor_copy(out=osl, in_=psum_t[:])
                        else:
                            nc.vector.tensor_tensor(
                                out=osl, in0=osl, in1=psum_t[:],
                                op=mybir.AluOpType.add)

            # ---- normalize and write out ----
            oa3 = out_acc[:].rearrange("p (s c) -> p s c", c=F + 1)
            sx = cpool.tile([128, nsb], f32)
            nc.vector.tensor_scalar_max(out=sx[:], in0=oa3[:, :, F],
                                        scalar1=1e-30)
            rs = cpool.tile([128, nsb], f32)
            nc.vector.reciprocal(out=rs[:], in_=sx[:])
            for sb in range(nsb):
                nc.vector.tensor_scalar_mul(
                    out=oa3[:, sb, 0:F], in0=oa3[:, sb, 0:F],
                    scalar1=rs[:, sb:sb + 1])
            nc.sync.dma_start(
                out=outp[:, :].rearrange("(s p) f -> p s f", p=128),
                in_=oa3[:, :, 0:F])
    nc.finalize()
    return nc


# --------------------------------------------------------------------------
# v1f: stall-free restructure of v1.
#   - er computed first; both phases' er ap_gathers issued early.
#   - ALL idx tiles (eridx, 2x16 elidx) pre-staged in dedicated tiles whose
#     DMAs complete during the el preamble -> no APGather waits on idx.
#   - el computed from the bf16 x stream (halves preamble DMA); el_d written
#     per src-half so ph0's el_rep broadcast starts ~100us in.
#   - 16 el gathers write 8 rotating disjoint elq buffers -> no 26.5us
#     GPSIMD semaphore-poll quantum per gather (the baseline lost ~900us
#     to that: every z-phase APGather waited on a dep at issue).
#   - rows pool deepened to prefetch row-gather chunks during the z-phase.
# --------------------------------------------------------------------------

def build_program_v1f(tpw, nt, t_s, n_src_pad=N_SRC_PAD, half=HALF,
                      dstpad=DSTPAD, nsb=NSB, nw=NW):
    f32 = mybir.dt.float32
    bf16 = mybir.dt.bfloat16
    i16 = mybir.dt.int16
    T = t_s
    Tc = T // 16

    nc = bacc.Bacc(None, target_bir_lowering=False,
                   dynamic_dma_scratch_size=_SCRATCH, num_swdge_queues=4)
    pool_order = [None]

    def pool_op(inst):
        if pool_order[0] is not None:
            add_dep_helper(inst.ins, pool_order[0].ins, False,
                           "pool library ordering")
        pool_order[0] = inst
        return inst

    xbf = nc.declare_dram_parameter("xbf", [n_src_pad, F], bf16, isOutput=False)
    xd = nc.declare_dram_parameter("xd", [dstpad, F], f32, isOutput=False)
    attnl = nc.declare_dram_parameter("attnl", [128, F], f32, isOutput=False)
    attnr = nc.declare_dram_parameter("attnr", [128, F], f32, isOutput=False)
    iota = nc.declare_dram_parameter("iota", [128, WIN], bf16, isOutput=False)
    pmask = nc.declare_dram_parameter("pmask", [128, 16], f32, isOutput=False)
    gidx = nc.declare_dram_parameter("gidx", [2, 128, 8 * T], i16, isOutput=False)
    elidx = nc.declare_dram_parameter("elidx", [2, 16, 128, Tc], i16, isOutput=False)
    eridx = nc.declare_dram_parameter("eridx", [2, 128, Tc], i16, isOutput=False)
    rel16 = nc.declare_dram_parameter("rel16", [2, 128, T], bf16, isOutput=False)
    outp = nc.declare_dram_parameter("out", [dstpad, F], f32, isOutput=True)
    el_d = nc.dram_tensor("el_d", [1, n_src_pad], f32)
    er_d = nc.dram_tensor("er_d", [1, dstpad], f32)

    nt_el = n_src_pad // 128          # 391
    nt_er = dstpad // 128             # 10
    nt_el_h0 = half // 128            # 196

    with tile.TileContext(nc) as tc:
        with (
            tc.tile_pool(name="const", bufs=1) as cpool,
            tc.tile_pool(name="big", bufs=1) as bigpool,
            tc.tile_pool(name="xt", bufs=2) as xtpool,
            tc.tile_pool(name="elq", bufs=6) as elqpool,
            tc.tile_pool(name="work", bufs=2) as wpool,
            tc.tile_pool(name="rows", bufs=4) as rpool,
            tc.tile_pool(name="sp", bufs=3) as sppool,
            tc.tile_pool(name="psum", bufs=4, space="PSUM") as ppool,
        ):
            # ---- constants + ALL idx tiles staged upfront ----
            attnl_sb = cpool.tile([128, F], f32)
            nc.sync.dma_start(out=attnl_sb[:], in_=attnl[:, :])
            attnr_sb = cpool.tile([128, F], f32)
            nc.sync.dma_start(out=attnr_sb[:], in_=attnr[:, :])
            iota_sb = cpool.tile([128, WIN], bf16)
            nc.sync.dma_start(out=iota_sb[:], in_=iota[:, :])
            pmask_sb = cpool.tile([128, 16], f32)
            nc.sync.dma_start(out=pmask_sb[:], in_=pmask[:, :])
            ones_sb = cpool.tile([128, 1], bf16)
            nc.vector.memset(ones_sb[:], 1.0)
            out_acc = cpool.tile([128, nsb * (F + 1)], f32)
            nc.vector.memset(out_acc[:], 1.0)
            junk = cpool.tile([128, F], f32)

            # one consolidated tile per idx family; single strided DMA each
            eridx_all = cpool.tile([128, 2, Tc], i16)
            nc.sync.dma_start(out=eridx_all[:],
                              in_=eridx[:, :, :].rearrange("a p t -> p a t"))
            elidx_all = cpool.tile([128, 32, Tc], i16)
            nc.sync.dma_start(
                out=elidx_all[:],
                in_=elidx[:, :, :, :].rearrange("a q p t -> p (a q) t"))
            rel_all = cpool.tile([128, 2, T], bf16)
            nc.sync.dma_start(out=rel_all[:],
                              in_=rel16[:, :, :].rearrange("a p t -> p a t"))

            # ---- er (first: small, unblocks both er gathers early) ----
            er_sb = cpool.tile([128, nt_er], f32)
            xdr2 = xd[:, :].rearrange("(t p) f -> p t f", p=128)
            xte = cpool.tile([128, nt_er, F], f32)
            nc.sync.dma_start(out=xte[:], in_=xdr2[:, :, :])
            for t in range(nt_er):
                nc.vector.scalar_tensor_tensor(
                    out=junk[:], in0=xte[:, t, :], scalar=1.0,
                    in1=attnr_sb[:],
                    op0=mybir.AluOpType.mult, op1=mybir.AluOpType.mult,
                    accum_out=er_sb[:, t:t + 1])
            nc.sync.dma_start(
                out=er_d[0, :].rearrange("(t p) -> p t", p=128), in_=er_sb[:])
            er_rep = cpool.tile([128, dstpad], f32)
            nc.sync.dma_start(out=er_rep[:],
                              in_=er_d[:, :].to_broadcast([128, dstpad]))

            er_edge = cpool.tile([128, 2, T], f32)
            for ph in range(2):
                pool_op(nc.gpsimd.ap_gather(
                    er_edge[:, ph, :], er_rep[:], eridx_all[:, ph, :],
                    channels=128, num_elems=dstpad, d=1, num_idxs=T))

            # ---- el from bf16 x stream; el_d written per half ----
            attnlb = cpool.tile([128, F], bf16)
            nc.vector.tensor_copy(out=attnlb[:], in_=attnl_sb[:])
            el_sb = cpool.tile([128, nt_el], f32)
            xbr2 = xbf[:, :].rearrange("(t p) f -> p t f", p=128)
            ELC16 = 16
            for hh in range(2):
                lo = 0 if hh == 0 else nt_el_h0
                hi = nt_el_h0 if hh == 0 else nt_el
                for t0 in range(lo, hi, ELC16):
                    ntc = min(ELC16, hi - t0)
                    xt = xtpool.tile([128, ELC16, F], bf16, tag="xtb")
                    nc.sync.dma_start(out=xt[:, :ntc, :],
                                      in_=xbr2[:, t0:t0 + ntc, :])
                    for i in range(ntc):
                        nc.vector.scalar_tensor_tensor(
                            out=junk[:], in0=xt[:, i, :], scalar=1.0,
                            in1=attnlb[:],
                            op0=mybir.AluOpType.mult, op1=mybir.AluOpType.mult,
                            accum_out=el_sb[:, t0 + i:t0 + i + 1])
                nt_h = (hi - lo)
                nc.sync.dma_start(
                    out=el_d[0, lo * 128:hi * 128].rearrange(
                        "(t p) -> p t", p=128),
                    in_=el_sb[:, lo:hi])

            # ---- phases ----
            ex_f = cpool.tile([128, T], f32)
            exb = cpool.tile([128, T], bf16)
            for ph in range(2):
                el_rep = bigpool.tile([128, half], f32, tag="el_rep")
                nc.sync.dma_start(
                    out=el_rep[:],
                    in_=el_d[:, ph * half:(ph + 1) * half].to_broadcast(
                        [128, half]))

                for q in range(16):
                    elq = elqpool.tile([128, T], f32, tag="elq")
                    pool_op(nc.gpsimd.ap_gather(
                        elq[:], el_rep[:], elidx_all[:, 16 * ph + q, :],
                        channels=128, num_elems=half, d=1, num_idxs=T))
                    nc.vector.scalar_tensor_tensor(
                        out=ex_f[:], in0=elq[:],
                        scalar=pmask_sb[:, q:q + 1],
                        in1=(er_edge[:, ph, :] if q == 0 else ex_f[:]),
                        op0=mybir.AluOpType.mult, op1=mybir.AluOpType.add)
                nc.vector.scalar_tensor_tensor(
                    out=ex_f[:], in0=ex_f[:], scalar=NEG_SLOPE, in1=ex_f[:],
                    op0=mybir.AluOpType.mult, op1=mybir.AluOpType.max)
                nc.scalar.activation(
                    out=ex_f[:], in_=ex_f[:],
                    func=mybir.ActivationFunctionType.Exp)
                nc.vector.tensor_copy(out=exb[:], in_=ex_f[:])

                # ---- main tile loop (unchanged structure) ----
                rows_t = None
                sp_t = None
                psum_t = None
                for t in range(nt):
                    ch, blk = divmod(t * TILE, CHUNK_E)
                    blk //= TILE
                    ib, ibc = divmod(ch, 16)
                    if blk == 0 and ibc == 0:
                        nei = min(16 * CHUNK_E, nt * TILE - ib * 16 * CHUNK_E)
                        gi_blk = wpool.tile([128, 16 * CHUNK_E // 16], i16,
                                            tag="gi")
                        nc.sync.dma_start(
                            out=gi_blk[:, :nei // 16],
                            in_=gidx[ph, :,
                                     ib * CHUNK_E:
                                     ib * CHUNK_E + nei // 16])
                    if blk == 0:
                        ne = min(CHUNK_E, nt * TILE - ch * CHUNK_E)
                        rows_t = rpool.tile([128, CHUNK_E // TILE, F], bf16,
                                            tag="rows")
                        pool_op(nc.gpsimd.dma_gather(
                            out_ap=rows_t[:, :ne // TILE, :],
                            in_ap=xbf[ph * half:(ph + 1) * half, :],
                            idxs_ap=gi_blk[:, ibc * (CHUNK_E // 16):
                                           ibc * (CHUNK_E // 16) + ne // 16],
                            num_idxs=ne, num_idxs_reg=ne, elem_size=F,
                            queue_num=ch % 4))
                    g, gblk = divmod(t, SG)
                    if gblk == 0:
                        sp_t = sppool.tile([128, SG * WIN], bf16, tag="sp")
                        gsl = slice(g * SG, min((g + 1) * SG, T))
                        nsg = gsl.stop - gsl.start
                        nc.vector.tensor_tensor(
                            out=sp_t[:, :nsg * WIN].rearrange(
                                "p (w o) -> p w o", o=WIN),
                            in0=iota_sb[:].rearrange(
                                "p (o w) -> p o w", o=1).to_broadcast(
                                    [128, nsg, WIN]),
                            in1=rel_all[:, ph, gsl].rearrange(
                                "p (w o) -> p w o", o=1).to_broadcast(
                                    [128, nsg, WIN]),
                            op=mybir.AluOpType.is_equal)
                        nc.vector.tensor_tensor(
                            out=sp_t[:, :nsg * WIN].rearrange(
                                "p (w o) -> p w o", o=WIN),
                            in0=sp_t[:, :nsg * WIN].rearrange(
                                "p (w o) -> p w o", o=WIN),
                            in1=exb[:, gsl].rearrange(
                                "p (w o) -> p w o", o=1).to_broadcast(
                                    [128, nsg, WIN]),
                            op=mybir.AluOpType.mult)
                    w, wt = divmod(t, tpw)
                    sb, wsb = divmod(w, 128 // WIN)
                    if wsb == 0 and wt == 0:
                        psum_t = ppool.tile([128, F + 1], f32, tag="acc")
                    bp = WIN * wsb
                    first = wt == 0
                    last = wt == tpw - 1
                    lhs = sp_t[:, gblk * WIN:(gblk + 1) * WIN]
                    nc.tensor.matmul(
                        out=psum_t[bp:bp + WIN, 0:F], lhsT=lhs,
                        rhs=rows_t[:, blk, :], start=first, stop=False,
                        tile_position=(0, bp))
                    nc.tensor.matmul(
                        out=psum_t[bp:bp + WIN, F:F + 1], lhsT=lhs,
                        rhs=ones_sb[:], start=False, stop=last,
                        tile_position=(0, bp))
                    if wsb == (128 // WIN) - 1 and last:
                        osl = out_acc[:, sb * (F + 1):(sb + 1) * (F + 1)]
                        if ph == 0:
                            nc.vector.tensor_copy(out=osl, in_=psum_t[:])
                        else:
                            nc.vector.tensor_tensor(
                                out=osl, in0=osl, in1=psum_t[:],
                                op=mybir.AluOpType.add)

            # ---- normalize and write out ----
            oa3 = out_acc[:].rearrange("p (s c) -> p s c", c=F + 1)
            sx = cpool.tile([128, nsb], f32)
            nc.vector.tensor_scalar_max(out=sx[:], in0=oa3[:, :, F],
                                        scalar1=1e-30)
            rs = cpool.tile([128, nsb], f32)
            nc.vector.reciprocal(out=rs[:], in_=sx[:])
            for sb in range(nsb):
                nc.vector.tensor_scalar_mul(
                    out=oa3[:, sb, 0:F], in0=oa3[:, sb, 0:F],
                    scalar1=rs[:, sb:sb + 1])
            nc.sync.dma_start(
                out=outp[:, :].rearrange("(s p) f -> p s f", p=128),
                in_=oa3[:, :, 0:F])
    nc.finalize()
    return nc


# --------------------------------------------------------------------------
# Entry point
# --------------------------------------------------------------------------

def _host_prep(x_src, x_dst, attn_l, attn_r, src_idx, dst_idx):
    x_src = np.asarray(x_src, np.float32).reshape(-1, F)
    x_dst = np.asarray(x_dst, np.float32).reshape(-1, F)
    al = np.asarray(attn_l, np.float32).reshape(F)
    ar = np.asarray(attn_r, np.float32).reshape(F)

    tpw, plans = build_plans(src_idx, dst_idx)
    x32p = np.zeros((N_SRC_PAD, F), np.float32)
    x32p[:N_SRC] = x_src
    xbfp = x32p.astype(BF16)
    attnl_bc = np.ascontiguousarray(np.broadcast_to(al, (128, F)))
    attnr_bc = np.ascontiguousarray(np.broadcast_to(ar, (128, F)))
    iota_np = np.broadcast_to(np.arange(WIN, dtype=np.float32),
                              (128, WIN)).astype(BF16)
    iota_np = np.ascontiguousarray(iota_np)
    pmask_np = np.zeros((128, 16), np.float32)
    pmask_np[np.arange(128), np.arange(128) % 16] = 1.0

    in_maps = []
    nt = t_s = None
    for c in range(N_CORES):
        per, nt, t_s = build_core_inputs(plans[c], tpw)
        xdp = np.zeros((DSTPAD, F), np.float32)
        xdp[:DPC] = x_dst[c * DPC:(c + 1) * DPC]
        in_maps.append({
            "x32": x32p, "xbf": xbfp, "xd": xdp,
            "attnl": attnl_bc, "attnr": attnr_bc, "iota": iota_np,
            "pmask": pmask_np,
            **per,
        })
    return tpw, nt, t_s, in_maps


_CACHE = {}


def _run(x_src, x_dst, attn_l, attn_r, src_idx, dst_idx, trace=False):
    from concourse.bass_utils import run_bass_kernel_spmd
    tpw, nt, t_s, in_maps = _host_prep(x_src, x_dst, attn_l, attn_r,
                                       src_idx, dst_idx)
    use_v1f = _os.environ.get("BASS_V1F", "1") == "1"
    key = ("v1f" if use_v1f else "v1", tpw, nt, t_s)
    if key not in _CACHE:
        _CACHE[key] = (build_program_v1f(tpw, nt, t_s) if use_v1f
                       else build_program(tpw, nt, t_s))
    nc = _CACHE[key]
    if use_v1f:
        in_maps = [{k: v for k, v in m.items() if k != "x32"}
                   for m in in_maps]
    res = run_bass_kernel_spmd(nc, in_maps, list(range(N_CORES)),
                               trace=trace)
    outs = [np.asarray(res.results[c]["out"])[:DPC] for c in range(N_CORES)]
    out = np.concatenate(outs, axis=0).reshape(N_DST, 1, F)
    return out.astype(np.float32), res


def kernel(x_src, x_dst, attn_l, attn_r, src_idx, dst_idx):
    out, _ = _run(x_src, x_dst, attn_l, attn_r, src_idx, dst_idx)
    return out


# --------------------------------------------------------------------------
# v2: no ap_gather, el packed into gathered rows, er via PE window matmuls
# --------------------------------------------------------------------------

def _plan_core_phase_v2(e_src, e_dst_local, nw, tpw):
    """Edge stream: per window, edges (dst-sorted) padded to tpw*TILE slots.
    Pads: g_idx=0, rel=-1. No per-dst padding."""
    n_slots = nw * tpw * TILE
    g_idx = np.zeros(n_slots, np.int32)
    rel = np.full(n_slots, -1, np.int32)
    cnt = np.bincount(e_dst_local, minlength=nw * WIN)
    wtot = np.add.reduceat(cnt, np.arange(nw) * WIN)
    assert (wtot <= tpw * TILE).all()
    win_base = np.arange(nw) * tpw * TILE
    e_cnt_start = np.concatenate([[0], np.cumsum(cnt)])[:-1]
    within_all = np.arange(len(e_src)) - np.repeat(
        np.concatenate([[0], np.cumsum(wtot)])[:-1], wtot)
    w_of_e = win_of = e_dst_local // WIN
    pos = win_base[win_of] + within_all
    g_idx[pos] = e_src
    rel[pos] = e_dst_local % WIN
    return g_idx, rel


def build_plans_v2(src_idx, dst_idx, n_cores=N_CORES, dpc=DPC, half=HALF,
                   nw=NW):
    src_idx = np.asarray(src_idx).astype(np.int64)
    dst_idx = np.asarray(dst_idx).astype(np.int64)
    bounds = np.searchsorted(dst_idx, np.arange(n_cores + 1) * dpc)
    split = []
    tpw = 1
    for c in range(n_cores):
        es = src_idx[bounds[c]:bounds[c + 1]].astype(np.int32)
        ed = (dst_idx[bounds[c]:bounds[c + 1]] - c * dpc).astype(np.int32)
        for ph in range(2):
            m = (es < half) if ph == 0 else (es >= half)
            ps, pd = es[m] - ph * half, ed[m]
            cnt = np.bincount(pd, minlength=nw * WIN)
            wt = np.add.reduceat(cnt, np.arange(nw) * WIN)
            tpw = max(tpw, int((wt.max() + TILE - 1) // TILE))
            split.append((c, ph, ps, pd))
    tpw = ((tpw + 3) // 4) * 4      # nt = nw*tpw must divide by SGT=32
    plans = [[None, None] for _ in range(n_cores)]
    for c, ph, ps, pd in split:
        plans[c][ph] = _plan_core_phase_v2(ps, pd, nw, tpw)
    return tpw, plans


def build_core_inputs_v2(plans_core, tpw, nw=NW):
    NT = nw * tpw
    T = NT
    gidx = np.zeros((2, 128, 8 * T), np.int16)
    rel16 = np.full((2, 128, T), -1, np.float32)
    srt = np.zeros((2, T, 32, TILE), np.float32)
    for ph in range(2):
        g, r = plans_core[ph]
        gidx[ph] = _wrap16(g)
        rT = r.reshape(T, TILE)
        rel16[ph] = rT.T
        srt[ph] = (np.arange(WIN)[None, :, None] == rT[:, None, :])
    return {
        "gidx": gidx,
        "rel16": rel16.astype(BF16),
        "srt": srt.astype(BF16),
    }, NT, T


CH2 = 2048   # edges per gather (v2)


def build_program_v2(tpw, nt, n_src_pad=N_SRC_PAD, half=HALF,
                     dstpad=DSTPAD, nsb=NSB, nw=NW):
    f32 = mybir.dt.float32
    bf16 = mybir.dt.bfloat16
    i16 = mybir.dt.int16
    T = nt
    XL = 256                  # XE row length in bf16 elems (512B)

    nc = bacc.Bacc(None, target_bir_lowering=False,
                   dynamic_dma_scratch_size=65536)
    x32 = nc.declare_dram_parameter("x32", [n_src_pad, F], f32, isOutput=False)
    xbf = nc.declare_dram_parameter("xbf", [n_src_pad, F], bf16, isOutput=False)
    xd = nc.declare_dram_parameter("xd", [dstpad, F], f32, isOutput=False)
    attnl = nc.declare_dram_parameter("attnl", [128, F], f32, isOutput=False)
    attnr = nc.declare_dram_parameter("attnr", [128, F], f32, isOutput=False)
    iota = nc.declare_dram_parameter("iota", [128, WIN], bf16, isOutput=False)
    gidx = nc.declare_dram_parameter("gidx", [2, 128, 8 * T], i16, isOutput=False)
    rel16 = nc.declare_dram_parameter("rel16", [2, 128, T], bf16, isOutput=False)
    srt = nc.declare_dram_parameter("srt", [2, T, 32, TILE], bf16, isOutput=False)
    outp = nc.declare_dram_parameter("out", [dstpad, F], f32, isOutput=True)
    el_d = nc.dram_tensor("el_d", [1, n_src_pad], f32)
    er_d = nc.dram_tensor("er_d", [1, dstpad], f32)
    xe = nc.dram_tensor("xe", [n_src_pad, XL], bf16)

    nt_el = n_src_pad // 128
    nt_er = dstpad // 128
    WPS = 128 // WIN          # windows per superblock
    SGT = 32 if nt % 32 == 0 else 16    # tiles per S'/er group

    with tile.TileContext(nc) as tc:
        with (
            tc.tile_pool(name="const", bufs=1) as cpool,
            tc.tile_pool(name="ph", bufs=2) as phpool,
            tc.tile_pool(name="work", bufs=2) as wpool,
            tc.tile_pool(name="rows", bufs=3) as rpool,
            tc.tile_pool(name="sp", bufs=2) as sppool,
            tc.tile_pool(name="srt", bufs=2) as srtpool,
            tc.tile_pool(name="psum", bufs=2, space="PSUM") as ppool,
            tc.tile_pool(name="erps", bufs=2, space="PSUM") as epool,
        ):
            # ---- constants ----
            attnl_sb = cpool.tile([128, F], f32)
            nc.sync.dma_start(out=attnl_sb[:], in_=attnl[:, :])
            attnr_sb = cpool.tile([128, F], f32)
            nc.sync.dma_start(out=attnr_sb[:], in_=attnr[:, :])
            iota_sb = cpool.tile([128, WIN], bf16)
            nc.sync.dma_start(out=iota_sb[:], in_=iota[:, :])
            ones_sb = cpool.tile([128, 1], bf16)
            nc.vector.memset(ones_sb[:], 1.0)
            out_acc = cpool.tile([128, nsb * (F + 1)], f32)
            junk = cpool.tile([128, F], f32)

            # ---- er ----
            er_sb = cpool.tile([128, nt_er], f32)
            xdr = xd[:, :].rearrange("(t p) f -> t p f", p=128)
            for t in range(nt_er):
                xt = wpool.tile([128, F], f32, tag="xt")
                nc.sync.dma_start(out=xt[:], in_=xdr[t])
                nc.vector.scalar_tensor_tensor(
                    out=junk[:], in0=xt[:], scalar=1.0, in1=attnr_sb[:],
                    op0=mybir.AluOpType.mult, op1=mybir.AluOpType.mult,
                    accum_out=er_sb[:, t:t + 1])
            nc.sync.dma_start(
                out=er_d[0, :].rearrange("(t p) -> p t", p=128), in_=er_sb[:])
            # window-major er [32, nw] + hi/lo bf16 split
            erw = cpool.tile([32, nw], f32)
            nc.sync.dma_start(
                out=erw[:], in_=er_d[0, :].rearrange("(w r) -> r w", r=32))
            erw_hi = cpool.tile([32, nw], bf16)
            nc.vector.tensor_copy(out=erw_hi[:], in_=erw[:])
            erw_hif = cpool.tile([32, nw], f32)
            nc.vector.tensor_copy(out=erw_hif[:], in_=erw_hi[:])
            erw_lof = cpool.tile([32, nw], f32)
            nc.vector.tensor_tensor(out=erw_lof[:], in0=erw[:],
                                    in1=erw_hif[:],
                                    op=mybir.AluOpType.subtract)
            erw_lo = cpool.tile([32, nw], bf16)
            nc.vector.tensor_copy(out=erw_lo[:], in_=erw_lof[:])
            erw_hl = cpool.tile([32, 2 * nw], bf16)
            nc.vector.tensor_copy(
                out=erw_hl[:].rearrange("p (w c) -> p w c", c=2)[:, :, 0:1],
                in_=erw_hi[:].rearrange("p (w c) -> p w c", c=1))
            nc.vector.tensor_copy(
                out=erw_hl[:].rearrange("p (w c) -> p w c", c=2)[:, :, 1:2],
                in_=erw_lo[:].rearrange("p (w c) -> p w c", c=1))


            # ---- el + XE build (per src half) ----
            # (XE pad bytes stay uninitialized: gathered but never consumed)
            el_sb = cpool.tile([128, nt_el], f32)
            x32r = x32[:, :].rearrange("(t p) f -> t p f", p=128)
            xe_f32 = xe[:, :].bitcast(f32)      # [n_src_pad, 128] f32 view
            for hh in range(2):
                for tt in range(nt_el // 2):
                    t = hh * (nt_el // 2) + tt
                    xt = wpool.tile([128, F], f32, tag="xt")
                    nc.sync.dma_start(out=xt[:], in_=x32r[t])
                    nc.vector.scalar_tensor_tensor(
                        out=junk[:], in0=xt[:], scalar=1.0, in1=attnl_sb[:],
                        op0=mybir.AluOpType.mult, op1=mybir.AluOpType.mult,
                        accum_out=el_sb[:, t:t + 1])
                # bf16 x into XE cols 0:128
                h0 = hh * (n_src_pad // 2)
                nc.sync.dma_start(
                    out=xe[h0:h0 + n_src_pad // 2, 0:F],
                    in_=xbf[h0:h0 + n_src_pad // 2, :])
                # el into XE (f32 view col 64)
                nc.sync.dma_start(
                    out=xe_f32[h0:h0 + n_src_pad // 2, 64:65].rearrange(
                        "(t p) o -> p t o", p=128),
                    in_=el_sb[:, hh * (nt_el // 2):(hh + 1) * (nt_el // 2)]
                    .rearrange("p (t o) -> p t o", o=1))

            # ---- phases ----
            assert nt % SGT == 0 and (SGT * TILE) % CH2 == 0
            chunks_per_group = SGT * TILE // CH2
            for ph in range(2):
                rel_sb = phpool.tile([128, T], bf16, tag="rel_sb")
                nc.sync.dma_start(out=rel_sb[:], in_=rel16[ph])
                exb = phpool.tile([128, T], bf16, tag="exb")

                psum_t = None
                for g in range(nt // SGT):
                    rows_g = []
                    for cc in range(chunks_per_group):
                        ch = g * chunks_per_group + cc
                        ib, ibc = divmod(ch, 8)
                        if ibc == 0:
                            nei = min(8 * CH2, nt * TILE - ib * 8 * CH2)
                            gi_blk = wpool.tile([128, 8 * CH2 // 16], i16,
                                                tag="gi")
                            nc.sync.dma_start(
                                out=gi_blk[:, :nei // 16],
                                in_=gidx[ph, :,
                                         ib * (8 * CH2 // 16):
                                         ib * (8 * CH2 // 16) + nei // 16])
                        rows_t = rpool.tile([128, CH2 // TILE, XL], bf16,
                                            tag="rows")
                        nc.gpsimd.dma_gather(
                            out_ap=rows_t[:],
                            in_ap=xe[ph * half:(ph + 1) * half, :],
                            idxs_ap=gi_blk[:, ibc * (CH2 // 16):
                                           (ibc + 1) * (CH2 // 16)],
                            num_idxs=CH2, num_idxs_reg=CH2, elem_size=XL)
                        rows_g.append(rows_t)
                    gsl = slice(g * SGT, (g + 1) * SGT)
                    # er matmuls (uploaded one-hot transposes @ er windows)
                    srt_t = srtpool.tile([32, SGT * TILE], bf16, tag="srt")
                    nc.sync.dma_start(
                        out=srt_t[:].rearrange("p (t e) -> p t e", e=TILE),
                        in_=srt[ph, gsl].rearrange("t r e -> r t e"))
                    erps_t = epool.tile([128, 2 * SGT], f32, tag="erps")
                    for gblk in range(SGT):
                        t = g * SGT + gblk
                        nc.tensor.matmul(
                            out=erps_t[:, 2 * gblk:2 * gblk + 2],
                            lhsT=srt_t[:, gblk * TILE:(gblk + 1) * TILE],
                            rhs=erw_hl[:, 2 * (t // tpw):2 * (t // tpw) + 2],
                            start=True, stop=True)
                    # S'raw = (iota == rel)
                    sp_t = sppool.tile([128, SGT * WIN], bf16, tag="sp")
                    nc.vector.tensor_tensor(
                        out=sp_t[:].rearrange("p (w o) -> p w o", o=WIN),
                        in0=iota_sb[:].rearrange(
                            "p (o w) -> p o w", o=1).to_broadcast(
                                [128, SGT, WIN]),
                        in1=rel_sb[:, gsl].rearrange(
                            "p (w o) -> p w o", o=1).to_broadcast(
                                [128, SGT, WIN]),
                        op=mybir.AluOpType.is_equal)
                    # z = el + er_hi + er_lo ; lrelu ; exp  (batched)
                    zg = wpool.tile([128, SGT], f32, tag="zg")
                    tpc = CH2 // TILE
                    for cc in range(chunks_per_group):
                        el_col = rows_g[cc][:, :, F:F + 2].bitcast(f32)
                        nc.vector.tensor_tensor(
                            out=zg[:, cc * tpc:(cc + 1) * tpc],
                            in0=el_col.rearrange("p a b -> p (a b)"),
                            in1=erps_t[:].rearrange(
                                "p (t c) -> p t c", c=2)[
                                :, cc * tpc:(cc + 1) * tpc, 0],
                            op=mybir.AluOpType.add)
                    nc.vector.tensor_tensor(
                        out=zg[:], in0=zg[:],
                        in1=erps_t[:].rearrange(
                            "p (t c) -> p t c", c=2)[:, :, 1],
                        op=mybir.AluOpType.add)
                    nc.vector.scalar_tensor_tensor(
                        out=zg[:], in0=zg[:], scalar=NEG_SLOPE, in1=zg[:],
                        op0=mybir.AluOpType.mult, op1=mybir.AluOpType.max)
                    nc.scalar.activation(
                        out=exb[:, gsl], in_=zg[:],
                        func=mybir.ActivationFunctionType.Exp)
                    # S' = S'raw * ex
                    nc.vector.tensor_tensor(
                        out=sp_t[:].rearrange("p (w o) -> p w o", o=WIN),
                        in0=sp_t[:].rearrange("p (w o) -> p w o", o=WIN),
                        in1=exb[:, gsl].rearrange(
                            "p (w o) -> p w o", o=1).to_broadcast(
                                [128, SGT, WIN]),
                        op=mybir.AluOpType.mult)
                    # segment matmuls
                    for gblk in range(SGT):
                        t = g * SGT + gblk
                        w, wt = divmod(t, tpw)
                        sb, wsb = divmod(w, WPS)
                        if wsb == 0 and wt == 0:
                            psum_t = ppool.tile([128, F + 1], f32, tag="acc")
                        bp = WIN * wsb
                        first = wt == 0
                        last = wt == tpw - 1
                        lhs = sp_t[:, gblk * WIN:(gblk + 1) * WIN]
                        cc, blk = divmod(gblk, tpc)
                        nc.tensor.matmul(
                            out=psum_t[bp:bp + WIN, 0:F], lhsT=lhs,
                            rhs=rows_g[cc][:, blk, 0:F], start=first,
                            stop=False, tile_position=(0, bp))
                        nc.tensor.matmul(
                            out=psum_t[bp:bp + WIN, F:F + 1], lhsT=lhs,
                            rhs=ones_sb[:], start=False, stop=last,
                            tile_position=(0, bp))
                        if wsb == WPS - 1 and last:
                            osl = out_acc[:,
                                          sb * (F + 1):(sb + 1) * (F + 1)]
                            if ph == 0:
                                nc.vector.tensor_copy(out=osl,
                                                      in_=psum_t[:])
                            else:
                                nc.vector.tensor_tensor(
                                    out=osl, in0=osl, in1=psum_t[:],
                                    op=mybir.AluOpType.add)

            # ---- normalize and write out ----
            oa3 = out_acc[:].rearrange("p (s c) -> p s c", c=F + 1)
            sx = cpool.tile([128, nsb], f32)
            nc.vector.tensor_scalar_max(out=sx[:], in0=oa3[:, :, F],
                                        scalar1=1e-30)
            rs = cpool.tile([128, nsb], f32)
            nc.vector.reciprocal(out=rs[:], in_=sx[:])
            for sb in range(nsb):
                nc.vector.tensor_scalar_mul(
                    out=oa3[:, sb, 0:F], in0=oa3[:, sb, 0:F],
                    scalar1=rs[:, sb:sb + 1])
            nc.sync.dma_start(
                out=outp[:, :].rearrange("(s p) f -> p s f", p=128),
                in_=oa3[:, :, 0:F])
    nc.finalize()
    return nc


def _host_prep_v2(x_src, x_dst, attn_l, attn_r, src_idx, dst_idx):
    x_src = np.asarray(x_src, np.float32).reshape(-1, F)
    x_dst = np.asarray(x_dst, np.float32).reshape(-1, F)
    al = np.asarray(attn_l, np.float32).reshape(F)
    ar = np.asarray(attn_r, np.float32).reshape(F)
    tpw, plans = build_plans_v2(src_idx, dst_idx)
    x32p = np.zeros((N_SRC_PAD, F), np.float32)
    x32p[:N_SRC] = x_src
    xbfp = x32p.astype(BF16)
    attnl_bc = np.ascontiguousarray(np.broadcast_to(al, (128, F)))
    attnr_bc = np.ascontiguousarray(np.broadcast_to(ar, (128, F)))
    iota_np = np.ascontiguousarray(
        np.broadcast_to(np.arange(WIN, dtype=np.float32),
                        (128, WIN))).astype(BF16)
    in_maps = []
    nt = None
    for c in range(N_CORES):
        per, nt, _ = build_core_inputs_v2(plans[c], tpw)
        xdp = np.zeros((DSTPAD, F), np.float32)
        xdp[:DPC] = x_dst[c * DPC:(c + 1) * DPC]
        in_maps.append({
            "x32": x32p, "xbf": xbfp, "xd": xdp,
            "attnl": attnl_bc, "attnr": attnr_bc, "iota": iota_np,
            **per,
        })
    return tpw, nt, in_maps


def _run_v2(x_src, x_dst, attn_l, attn_r, src_idx, dst_idx, trace=False):
    from concourse.bass_utils import run_bass_kernel_spmd
    tpw, nt, in_maps = _host_prep_v2(x_src, x_dst, attn_l, attn_r,
                                     src_idx, dst_idx)
    key = ("v2", tpw, nt)
    if key not in _CACHE:
        _CACHE[key] = build_program_v2(tpw, nt)
    nc = _CACHE[key]
    res = run_bass_kernel_spmd(nc, in_maps, list(range(N_CORES)),
                               trace=trace)
    outs = [np.asarray(res.results[c]["out"])[:DPC] for c in range(N_CORES)]
    out = np.concatenate(outs, axis=0).reshape(N_DST, 1, F)
    return out.astype(np.float32), res


# --------------------------------------------------------------------------
# v3: scatter-W design — no per-edge DMA. W[src%128, dst] built per 128-src
# chunk via GPSIMD local_scatter of bf16 attention values; out accumulated
# as x_c.T @ W_c on PE across 391 chunks; sum_ex from bf16 W accumulator.
# --------------------------------------------------------------------------

NCH = N_SRC_PAD // 128          # 391 src chunks
NBLK = 4                        # z-pipeline blocks


def _prep_v3(src_idx, dst_idx):
    src = np.asarray(src_idx).astype(np.int64)
    dst = np.asarray(dst_idx).astype(np.int64)
    bounds = np.searchsorted(dst, np.arange(N_CORES + 1) * DPC)
    per_core = []
    cnts = np.zeros((N_CORES, NCH * 128), np.int64)
    for c8 in range(N_CORES):
        s = src[bounds[c8]:bounds[c8 + 1]]
        v = dst[bounds[c8]:bounds[c8 + 1]] - c8 * DPC
        key = s * 2048 + v
        uk, kc = np.unique(key, return_counts=True)
        us = (uk >> 11).astype(np.int64)
        uv = (uk & 2047).astype(np.int64)
        cnts[c8] = np.bincount(us, minlength=NCH * 128)
        per_core.append((us, uv, kc))
    ncell = cnts.max(0).reshape(NCH, 128).max(1)
    n_c = np.maximum(((ncell + 1) // 2) * 2, 2).astype(np.int64)
    bl = [0, 98, 196, 294, NCH]
    off = np.zeros(NCH, np.int64)
    col_blocks = []
    cur = 0
    for b in range(NBLK):
        cs = cur
        for c in range(bl[b], bl[b + 1]):
            off[c] = cur
            cur += n_c[c]
        cur = ((cur + 15) // 16) * 16
        col_blocks.append((cs, cur))
    T = cur

    ch_of_col = np.zeros(T, np.int64)
    for c in range(NCH):
        ch_of_col[off[c]:off[c] + n_c[c]] = c
    a = ch_of_col.reshape(T // 16, 16).T.astype(np.int16)
    elidx = np.tile(a, (8, 1))

    streams = []
    for c8 in range(N_CORES):
        us, uv, kc = per_core[c8]
        order = np.argsort(us, kind="stable")
        us_o, uv_o, kc_o = us[order], uv[order], kc[order]
        cellcnt = np.bincount(us_o, minlength=NCH * 128)
        starts = np.concatenate([[0], np.cumsum(cellcnt)])[:-1]
        j = np.arange(len(us_o)) - starts[us_o]
        cols = off[us_o // 128] + j
        rows = us_o % 128
        scat = np.full((128, T), -1, np.int16)
        scat[rows, cols] = uv_o.astype(np.int16)
        kfac = np.zeros((128, T), np.float32)
        kfac[rows, cols] = kc_o
        vfull = np.where(scat >= 0, scat, 0).astype(np.int16)
        eridx = np.zeros((16, 128, T // 16), np.int16)
        for q in range(16):
            sm = vfull[q::16, :]
            aa = sm.reshape(8, T // 16, 16).transpose(0, 2, 1)
            eridx[q] = aa.reshape(128, T // 16)
        streams.append({"scat": scat, "kfac": kfac.astype(BF16),
                        "eridx": eridx})
    return T, n_c, off, bl, col_blocks, elidx, streams


def build_program_v3(T, n_c, off, bl, col_blocks):
    f32 = mybir.dt.float32
    bf16 = mybir.dt.bfloat16
    i16 = mybir.dt.int16

    nc = bacc.Bacc(None, target_bir_lowering=False)
    xdev = nc.declare_dram_parameter("xdev", [128, NCH * F], bf16, isOutput=False)
    xddev = nc.declare_dram_parameter("xddev", [128, NSB * F], f32, isOutput=False)
    attnl = nc.declare_dram_parameter("attnl", [128, F], bf16, isOutput=False)
    attnr = nc.declare_dram_parameter("attnr", [128, F], f32, isOutput=False)
    pmask = nc.declare_dram_parameter("pmask", [128, 16], f32, isOutput=False)
    scat = nc.declare_dram_parameter("scat", [128, T], i16, isOutput=False)
    kfac = nc.declare_dram_parameter("kfac", [128, T], bf16, isOutput=False)
    eridx = nc.declare_dram_parameter("eridx", [16, 128, T // 16], i16, isOutput=False)
    elidx = nc.declare_dram_parameter("elidx", [128, T // 16], i16, isOutput=False)
    outp = nc.declare_dram_parameter("out", [128, DSTPAD], f32, isOutput=True)
    er_d = nc.dram_tensor("er_d", [1, DSTPAD], f32)
    sums_d = nc.dram_tensor("sums_d", [1, DSTPAD], f32)

    bTmax = max(ce - cs for cs, ce in col_blocks)

    with tile.TileContext(nc) as tc:
        with (
            tc.tile_pool(name="const", bufs=1) as cpool,
            tc.tile_pool(name="z", bufs=1) as zpool,
